# revision 1
# baseline (speedup 1.0000x reference)
"""GroupQueryAttention on 8 TRN2 NeuronCores.

Strategy: tensor-parallel over heads. H=32 query heads, KV=8 kv heads,
group size G=4 -> each core owns exactly 1 kv head and its 4 query heads.
Per core:
  - QKV projections from a replicated (pre-transposed, channels-major) input
  - RoPE on Q/K (rotate-half, done on DVE across partition halves)
  - attention with scores computed TRANSPOSED ([keys, q] layout) so the
    exp(scores) tiles feed the V-matmul directly as the moving operand
    (no P-transpose needed); softmax normalization is deferred: O = V.E,
    then ctx = O * (1/colsum(E)) broadcast via a rank-1 matmul
  - partial output ctx @ Wo_shard  (row-shard of Wo)
Host sums the 8 partial outputs (the "all-reduce" of the row-parallel Wo).

Causal mask: the attention_mask input is verified on host to be the
upper-triangular causal mask; the device program exploits causality by
skipping fully-masked key-tiles and applying 0/1 mask tiles on the 4
diagonal-crossing key-tiles of each query chunk. If the mask is ever not
causal, a numpy fallback computes the exact reference on host.

Compute dtype: bf16 on the PE (f32 PSUM accumulation), f32 RoPE/softmax
bookkeeping. Output partials returned as bf16, summed on host in f32.
"""

import sys

sys.path.insert(0, "/opt/trn_rl_repo")

from contextlib import ExitStack

import numpy as np
import ml_dtypes

import concourse.bass as bass
import concourse.bacc as bacc
import concourse.tile as tile
from concourse import mybir
from concourse.bass_utils import run_bass_kernel_spmd

BF16 = ml_dtypes.bfloat16

S = 2048          # sequence length
DIN = 4096        # model dim
H, KV, DH = 32, 8, 128
G = H // KV       # 4 query heads per kv head
NCORES = 8
HPC = H // NCORES     # 4 query heads per core
DPC = HPC * DH        # 512 = per-core q-projection width

NQ = 4            # s-quarters (chunks of 512 queries)
QC = S // NQ      # 512
KT = 128          # key tile (partition dim of transposed scores)
NKT = S // KT     # 16 key tiles
NK = DIN // 128   # 32 contraction tiles for projections
SCALE = 1.0 / float(np.sqrt(DH))
EXP_BIAS = -10.0  # constant shift inside exp; cancels in normalization


def build_nc():
    """Build the per-core Bass program (same program on all 8 cores; the
    per-core weight shards arrive via in_maps)."""
    nc = bacc.Bacc()
    dt = mybir.dt

    # ---- DRAM parameters (host-prepared layouts; all DMA-contiguous) ----
    # x[p, sq, k, sc] = x_orig[512*sq + sc, 128*k + p]   (channels-major)
    x = nc.declare_dram_parameter("x", [128, NQ, NK, QC], dt.bfloat16, isOutput=False)
    # wq[p, k, m, d] = Wq_shard[128*k + p, 128*m + d]
    wq = nc.declare_dram_parameter("wq", [128, NK, HPC, DH], dt.bfloat16, isOutput=False)
    # wk[p, k, d] = Wk_shard[128*k + p, d]
    wk = nc.declare_dram_parameter("wk", [128, NK, DH], dt.bfloat16, isOutput=False)
    wv = nc.declare_dram_parameter("wv", [128, NK, DH], dt.bfloat16, isOutput=False)
    # wo[p, h, n] = Wo_shard[128*h + p, n]
    wo = nc.declare_dram_parameter("wo", [128, HPC, DIN], dt.bfloat16, isOutput=False)
    # cosT[d, s] = cos[s, d]; sinm[d, s] = -sin[s, d] for d<64 else +sin[s, d]
    cosT = nc.declare_dram_parameter("cosT", [DH, S], dt.float32, isOutput=False)
    sinm = nc.declare_dram_parameter("sinm", [DH, S], dt.float32, isOutput=False)
    # m01[p, r, f] = 0.0 where 128*r + p > f else 1.0  (diagonal-tile masks)
    m01 = nc.declare_dram_parameter("m01", [128, 4, QC], dt.bfloat16, isOutput=False)
    ident = nc.declare_dram_parameter("ident", [128, 128], dt.bfloat16, isOutput=False)
    ones_col = nc.declare_dram_parameter("ones_col", [128, 1], dt.bfloat16, isOutput=False)
    ones_row = nc.declare_dram_parameter("ones_row", [1, 128], dt.float32, isOutput=False)
    out = nc.declare_dram_parameter("out", [S, DIN], dt.bfloat16, isOutput=True)

    with tile.TileContext(nc) as tc, ExitStack() as ctx:
        singles = ctx.enter_context(tc.tile_pool(name="singles", bufs=1))
        wpool = ctx.enter_context(tc.tile_pool(name="wpool", bufs=1))
        xpool = ctx.enter_context(tc.tile_pool(name="xpool", bufs=2))
        qkv = ctx.enter_context(tc.tile_pool(name="qkv", bufs=1))
        epool = ctx.enter_context(tc.tile_pool(name="epool", bufs=6))
        spool = ctx.enter_context(tc.tile_pool(name="spool", bufs=3))
        tpool = ctx.enter_context(tc.tile_pool(name="tpool", bufs=2))
        obuf = ctx.enter_context(tc.tile_pool(name="obuf", bufs=4))
        ps_acc = ctx.enter_context(tc.tile_pool(name="ps_acc", bufs=2, space="PSUM"))
        ps_sc = ctx.enter_context(tc.tile_pool(name="ps_sc", bufs=4, space="PSUM"))
        ps_sm = ctx.enter_context(tc.tile_pool(name="ps_sm", bufs=2, space="PSUM"))

        # ---- constants / weights resident in SBUF ----
        # DMA emission order matters: the K-projection consumes w_k and the
        # first x sub-tiles, so those go first; everything else follows in
        # consumption order to keep the PE from stalling at kernel start.
        w_k = singles.tile([128, NK, DH], dt.bfloat16, tag="wk")
        nc.sync.dma_start(out=w_k, in_=wk[:])

        x_t0 = xpool.tile([128, NK, QC], dt.bfloat16, tag="xq", name="x_t0")
        for g in range(4):
            nc.sync.dma_start(out=x_t0[:, g * 8:(g + 1) * 8],
                              in_=x[:, 0, g * 8:(g + 1) * 8])

        c_cos = singles.tile([DH, S], dt.float32, tag="cos")
        nc.sync.dma_start(out=c_cos, in_=cosT[:])
        c_sin = singles.tile([DH, S], dt.float32, tag="sin")
        nc.sync.dma_start(out=c_sin, in_=sinm[:])

        w_q = wpool.tile([128, NK, HPC, DH], dt.bfloat16, tag="wbig")
        for h in range(HPC):
            nc.sync.dma_start(out=w_q[:, :, h], in_=wq[:, :, h])
        w_v = singles.tile([128, NK, DH], dt.bfloat16, tag="wv")
        nc.sync.dma_start(out=w_v, in_=wv[:])

        c_m01 = singles.tile([128, 4, QC], dt.bfloat16, tag="m01")
        nc.sync.dma_start(out=c_m01, in_=m01[:])
        c_id = singles.tile([128, 128], dt.bfloat16, tag="ident")
        nc.sync.dma_start(out=c_id, in_=ident[:])
        c_oc = singles.tile([128, 1], dt.bfloat16, tag="ones_col")
        nc.sync.dma_start(out=c_oc, in_=ones_col[:])
        c_or = singles.tile([1, 128], dt.float32, tag="ones_row")
        nc.sync.dma_start(out=c_or, in_=ones_row[:])
        c_bias = singles.tile([128, 1], dt.float32, tag="ebias")
        nc.vector.memset(c_bias, EXP_BIAS)

        # ---- long-lived activations ----
        qt = [qkv.tile([DH, S], dt.bfloat16, tag=f"qt{h}", name=f"qt{h}") for h in range(HPC)]
        kt = qkv.tile([DH, S], dt.bfloat16, tag="kt")
        vn = qkv.tile([128, NKT, DH], dt.bfloat16, tag="vn")     # V natural [j, d] tiles
        ctxT = [qkv.tile([DH, S], dt.bfloat16, tag=f"ctx{h}", name=f"ctx{h}") for h in range(HPC)]

        def rope_from_psum(ps, dst_slice, s0):
            """dst = ps*cos + rot_half(ps)*sinm over s-columns [s0, s0+QC)."""
            t1 = tpool.tile([DH, QC], dt.float32, tag="t1", name="t1")
            nc.vector.tensor_mul(t1, ps, c_cos[:, s0:s0 + QC])
            t2 = tpool.tile([DH, QC], dt.float32, tag="t2", name="t2")
            nc.vector.tensor_mul(t2[0:64, :], ps[64:128, :], c_sin[0:64, s0:s0 + QC])
            nc.vector.tensor_mul(t2[64:128, :], ps[0:64, :], c_sin[64:128, s0:s0 + QC])
            nc.vector.tensor_add(dst_slice, t1, t2)

        for sq in range(NQ):
            s0 = sq * QC
            if sq == 0:
                x_t = x_t0
            else:
                x_t = xpool.tile([128, NK, QC], dt.bfloat16, tag="xq", name="x_t")
                for g in range(4):
                    nc.sync.dma_start(out=x_t[:, g * 8:(g + 1) * 8],
                                      in_=x[:, sq, g * 8:(g + 1) * 8])

            # K projection + RoPE
            psk = ps_acc.tile([DH, QC], dt.float32, tag="acc", name="psk")
            for k in range(NK):
                nc.tensor.matmul(psk, lhsT=w_k[:, k], rhs=x_t[:, k],
                                 start=(k == 0), stop=(k == NK - 1))
            rope_from_psum(psk, kt[:, s0:s0 + QC], s0)

            # Q projections + RoPE
            for h in range(HPC):
                psq = ps_acc.tile([DH, QC], dt.float32, tag="acc", name="psq")
                for k in range(NK):
                    nc.tensor.matmul(psq, lhsT=w_q[:, k, h], rhs=x_t[:, k],
                                     start=(k == 0), stop=(k == NK - 1))
                rope_from_psum(psq, qt[h][:, s0:s0 + QC], s0)

            # V projection (transposed layout), then PE-transpose to natural
            psv = ps_acc.tile([DH, QC], dt.float32, tag="acc", name="psv")
            for k in range(NK):
                nc.tensor.matmul(psv, lhsT=w_v[:, k], rhs=x_t[:, k],
                                 start=(k == 0), stop=(k == NK - 1))
            vtmp = tpool.tile([DH, QC], dt.bfloat16, tag="vtmp", name="vtmp")
            nc.scalar.copy(vtmp, psv)
            for i in range(QC // 128):
                pvt = ps_sc.tile([128, 128], dt.bfloat16, tag="sc", name="pvt")
                nc.tensor.transpose(pvt, vtmp[:, i * 128:(i + 1) * 128], c_id)
                nc.scalar.copy(vn[:, sq * 4 + i], pvt)

            # ---- attention for this quarter's queries (causal) ----
            njt = 4 * (sq + 1)
            for h in range(HPC):
                sacc = spool.tile([128, QC], dt.bfloat16, tag="sacc", name="sacc")
                nc.vector.memset(sacc, 0.0)
                po = ps_acc.tile([DH, QC], dt.float32, tag="acc", name="po")
                for jt in range(njt):
                    psc = ps_sc.tile([128, QC], dt.float32, tag="sc", name="psc")
                    nc.tensor.matmul(psc, lhsT=kt[:, jt * KT:(jt + 1) * KT],
                                     rhs=qt[h][:, s0:s0 + QC], start=True, stop=True)
                    e = epool.tile([128, QC], dt.bfloat16, tag="e", name="e")
                    nc.scalar.activation(out=e, in_=psc,
                                         func=mybir.ActivationFunctionType.Exp,
                                         bias=c_bias, scale=SCALE)
                    r = jt - (njt - 4)
                    if r >= 0:
                        nc.vector.tensor_mul(e, e, c_m01[:, r])
                    nc.vector.tensor_add(sacc, sacc, e)
                    nc.tensor.matmul(po, lhsT=vn[:, jt], rhs=e,
                                     start=(jt == 0), stop=(jt == njt - 1))
                # normalization: ctx = O * (1 / colsum(E)) broadcast over d
                pcs = ps_sm.tile([1, QC], dt.float32, tag="sm", name="pcs")
                nc.tensor.matmul(pcs, lhsT=c_oc, rhs=sacc, start=True, stop=True)
                rec = tpool.tile([1, QC], dt.float32, tag="rec", name="rec")
                nc.vector.reciprocal(rec, pcs)
                prb = ps_sm.tile([128, QC], dt.float32, tag="sm", name="prb")
                nc.tensor.matmul(prb, lhsT=c_or, rhs=rec, start=True, stop=True)
                rbs = tpool.tile([128, QC], dt.float32, tag="rbs", name="rbs")
                nc.scalar.copy(rbs, prb)
                nc.vector.tensor_mul(ctxT[h][:, s0:s0 + QC], po, rbs)

        # ---- output projection: out[s, n] += ctxT[h].T @ Wo ----
        w_o = wpool.tile([128, HPC, DIN], dt.bfloat16, tag="wbig", name="w_o")
        nc.sync.dma_start(out=w_o, in_=wo[:])
        for st in range(S // 128):
            for oc in range(DIN // 512):
                pso = ps_acc.tile([128, 512], dt.float32, tag="acc", name="pso")
                for h in range(HPC):
                    nc.tensor.matmul(pso, lhsT=ctxT[h][:, st * 128:(st + 1) * 128],
                                     rhs=w_o[:, h, oc * 512:(oc + 1) * 512],
                                     start=(h == 0), stop=(h == HPC - 1))
                ob = obuf.tile([128, 512], dt.bfloat16, tag="ob", name="ob")
                nc.scalar.copy(ob, pso)
                nc.sync.dma_start(out=out[st * 128:(st + 1) * 128, oc * 512:(oc + 1) * 512],
                                  in_=ob)
    nc.finalize()
    return nc


def make_in_maps(input_tensor, cos, sin, Wq, Wk, Wv, Wo):
    """Host-side sharding + layout preparation. Returns list of 8 dicts."""
    x2 = np.ascontiguousarray(input_tensor.reshape(S, DIN))
    # x_host[p, sq, k, sc] = x2[512*sq+sc, 128*k+p]
    xt = x2.T.astype(BF16)                      # [DIN, S]
    x_host = np.ascontiguousarray(
        xt.reshape(NK, 128, NQ, QC).transpose(1, 2, 0, 3))

    cosT = np.ascontiguousarray(cos.T.astype(np.float32))
    sinm = np.ascontiguousarray(sin.T.astype(np.float32))
    sinm = sinm.copy()
    sinm[0:64, :] *= -1.0

    p_idx = np.arange(128)[:, None, None]
    r_idx = np.arange(4)[None, :, None]
    f_idx = np.arange(QC)[None, None, :]
    m01 = ((128 * r_idx + p_idx) <= f_idx).astype(BF16)

    ident = np.eye(128, dtype=BF16)
    ones_col = np.ones((128, 1), dtype=BF16)
    ones_row = np.ones((1, 128), dtype=np.float32)

    common = dict(x=x_host, cosT=cosT, sinm=sinm, m01=m01, ident=ident,
                  ones_col=ones_col, ones_row=ones_row)

    in_maps = []
    for c in range(NCORES):
        wq_s = Wq[:, c * DPC:(c + 1) * DPC].astype(BF16)
        wq_host = np.ascontiguousarray(
            wq_s.reshape(NK, 128, HPC, DH).transpose(1, 0, 2, 3))
        wk_s = Wk[:, c * DH:(c + 1) * DH].astype(BF16)
        wk_host = np.ascontiguousarray(wk_s.reshape(NK, 128, DH).transpose(1, 0, 2))
        wv_s = Wv[:, c * DH:(c + 1) * DH].astype(BF16)
        wv_host = np.ascontiguousarray(wv_s.reshape(NK, 128, DH).transpose(1, 0, 2))
        wo_s = Wo[c * DPC:(c + 1) * DPC, :].astype(BF16)
        wo_host = np.ascontiguousarray(wo_s.reshape(HPC, 128, DIN).transpose(1, 0, 2))
        in_maps.append(dict(common, wq=wq_host, wk=wk_host, wv=wv_host, wo=wo_host))
    return in_maps


def _numpy_fallback(input_tensor, attention_mask, cos, sin, Wq, Wk, Wv, Wo):
    x = input_tensor.astype(np.float32)
    b, s, _ = x.shape
    q = (x @ Wq).reshape(b, s, H, DH).transpose(0, 2, 1, 3)
    k = (x @ Wk).reshape(b, s, KV, DH).transpose(0, 2, 1, 3)
    v = (x @ Wv).reshape(b, s, KV, DH).transpose(0, 2, 1, 3)

    def rope(t):
        t1, t2 = t[..., :64], t[..., 64:]
        rot = np.concatenate([-t2, t1], axis=-1)
        return t * cos[None, None] + rot * sin[None, None]

    q, k = rope(q), rope(k)
    k = np.repeat(k, G, axis=1)
    v = np.repeat(v, G, axis=1)
    sc = np.einsum('bhqd,bhkd->bhqk', q, k)
    sc = np.where(attention_mask, -np.inf, sc) / np.float32(np.sqrt(DH))
    sc = sc - sc.max(axis=-1, keepdims=True)
    w = np.exp(sc)
    w = w / w.sum(axis=-1, keepdims=True)
    ctx = np.einsum('bhqk,bhkd->bhqd', w, v)
    ctx = ctx.transpose(0, 2, 1, 3).reshape(b, s, H * DH)
    return (ctx @ Wo).astype(np.float32)


_NC_CACHE = {}


def kernel(input_tensor, attention_mask, cos, sin, Wq, Wk, Wv, Wo):
    mask = np.asarray(attention_mask).reshape(S, S)
    causal = np.array_equal(mask, np.triu(np.ones((S, S), bool), k=1))
    if not causal:
        return _numpy_fallback(np.asarray(input_tensor), np.asarray(attention_mask),
                               np.asarray(cos), np.asarray(sin),
                               np.asarray(Wq), np.asarray(Wk),
                               np.asarray(Wv), np.asarray(Wo))

    if "nc" not in _NC_CACHE:
        _NC_CACHE["nc"] = build_nc()
    nc = _NC_CACHE["nc"]

    in_maps = make_in_maps(np.asarray(input_tensor), np.asarray(cos),
                           np.asarray(sin), np.asarray(Wq), np.asarray(Wk),
                           np.asarray(Wv), np.asarray(Wo))
    res = run_bass_kernel_spmd(nc, in_maps, core_ids=list(range(NCORES)))
    acc = np.zeros((S, DIN), np.float32)
    for r in res.results:
        acc += np.asarray(r["out"], dtype=np.float32)
    return acc.reshape(1, S, DIN)



# revision 2
# speedup vs baseline: 1.0546x; 1.0546x over previous
"""GroupQueryAttention on 8 TRN2 NeuronCores.

Strategy: tensor-parallel over heads. H=32 query heads, KV=8 kv heads,
group size G=4 -> each core owns exactly 1 kv head and its 4 query heads.
Per core:
  - QKV projections from a replicated (pre-transposed, channels-major) input
  - RoPE on Q/K (rotate-half, done on DVE across partition halves)
  - attention with scores computed TRANSPOSED ([keys, q] layout) so the
    exp(scores) tiles feed the V-matmul directly as the moving operand;
    softmax normalization is deferred: O = V.E, then ctx = O * (1/colsum(E))
  - partial output ctx @ Wo_shard  (row-shard of Wo)
Host sums the 8 partial outputs (the "all-reduce" of the row-parallel Wo).

Perf structure (v2):
  - softmax colsum via gpsimd partition_all_reduce (idle engine) and the
    reciprocal via ACT ln -> exp(-x) (same activation table set as the
    score exps), so the PE never waits on normalization
  - causal diagonal tiles trimmed: score/PV matmuls only cover the
    not-fully-masked query columns; a single 128x128 lower-tri mask
  - out-projection of quarter q-1 is emitted interleaved into quarter q's
    attention so the PE has independent matmuls to chew on while exp
    results are pending
  - PSUM->SBUF drains split between ACT and DVE; output rows coalesced
    into [128, 2048] tiles before DMA
"""

import sys

sys.path.insert(0, "/opt/trn_rl_repo")

from contextlib import ExitStack

import numpy as np
import ml_dtypes

import concourse.bass as bass
import concourse.bacc as bacc
import concourse.tile as tile
from concourse import mybir
from concourse import bass_isa
from concourse.bass_utils import run_bass_kernel_spmd

BF16 = ml_dtypes.bfloat16

S = 2048          # sequence length
DIN = 4096        # model dim
H, KV, DH = 32, 8, 128
G = H // KV       # 4 query heads per kv head
NCORES = 8
HPC = H // NCORES     # 4 query heads per core
DPC = HPC * DH        # 512 = per-core q-projection width

NQ = 4            # s-quarters (chunks of 512 queries)
QC = S // NQ      # 512
KT = 128          # key tile (partition dim of transposed scores)
NKT = S // KT     # 16 key tiles
NK = DIN // 128   # 32 contraction tiles for projections
NXC = 4           # x chunks per quarter (k-groups of 8)
SCALE = 1.0 / float(np.sqrt(DH))
EXP_BIAS = -10.0  # constant shift inside exp; cancels in normalization


def build_nc():
    """Build the per-core Bass program (same program on all 8 cores; the
    per-core weight shards arrive via in_maps)."""
    nc = bacc.Bacc()
    dt = mybir.dt

    # ---- DRAM parameters (host-prepared layouts; all DMA-contiguous) ----
    # x[p, sq, k, sc] = x_orig[512*sq + sc, 128*k + p]   (channels-major)
    x = nc.declare_dram_parameter("x", [128, NQ, NK, QC], dt.bfloat16, isOutput=False)
    # wq[p, k, m, d] = Wq_shard[128*k + p, 128*m + d]
    wq = nc.declare_dram_parameter("wq", [128, NK, HPC, DH], dt.bfloat16, isOutput=False)
    # wk[p, k, d] = Wk_shard[128*k + p, d]
    wk = nc.declare_dram_parameter("wk", [128, NK, DH], dt.bfloat16, isOutput=False)
    wv = nc.declare_dram_parameter("wv", [128, NK, DH], dt.bfloat16, isOutput=False)
    # wo[p, h, n] = Wo_shard[128*h + p, n]
    wo = nc.declare_dram_parameter("wo", [128, HPC, DIN], dt.bfloat16, isOutput=False)
    # cosT[d, s] = cos[s, d]; sinm[d, s] = -sin[s, d] for d<64 else +sin[s, d]
    cosT = nc.declare_dram_parameter("cosT", [DH, S], dt.float32, isOutput=False)
    sinm = nc.declare_dram_parameter("sinm", [DH, S], dt.float32, isOutput=False)
    # tri[p, c] = 1.0 if p <= c else 0.0  (128x128 causal triangle)
    tri = nc.declare_dram_parameter("tri", [128, 128], dt.bfloat16, isOutput=False)
    ident = nc.declare_dram_parameter("ident", [128, 128], dt.bfloat16, isOutput=False)
    out = nc.declare_dram_parameter("out", [S, DIN], dt.bfloat16, isOutput=True)

    with tile.TileContext(nc) as tc, ExitStack() as ctx:
        singles = ctx.enter_context(tc.tile_pool(name="singles", bufs=1))
        wqp = ctx.enter_context(tc.tile_pool(name="wqp", bufs=1))
        wop = ctx.enter_context(tc.tile_pool(name="wop", bufs=1))
        xp = ctx.enter_context(tc.tile_pool(name="xp", bufs=1))
        qkv = ctx.enter_context(tc.tile_pool(name="qkv", bufs=1))
        epool = ctx.enter_context(tc.tile_pool(name="epool", bufs=4))
        spool = ctx.enter_context(tc.tile_pool(name="spool", bufs=2))
        npool = ctx.enter_context(tc.tile_pool(name="npool", bufs=4))
        tpool = ctx.enter_context(tc.tile_pool(name="tpool", bufs=2))
        obp = ctx.enter_context(tc.tile_pool(name="obp", bufs=2))
        ps_proj = ctx.enter_context(tc.tile_pool(name="ps_proj", bufs=2, space="PSUM"))
        ps_po = ctx.enter_context(tc.tile_pool(name="ps_po", bufs=2, space="PSUM"))
        ps_sc = ctx.enter_context(tc.tile_pool(name="ps_sc", bufs=2, space="PSUM"))
        ps_out = ctx.enter_context(tc.tile_pool(name="ps_out", bufs=2, space="PSUM"))

        # ---- constants / weights resident in SBUF ----
        # DMA emission order == consumption order so the PE never stalls at
        # kernel start: wk + first x chunk feed the K-projection; cos/sin
        # feed RoPE right after; wq/wv follow; wo is needed only once
        # quarter-0 attention finishes.
        w_k = singles.tile([128, NK, DH], dt.bfloat16, tag="wk")
        nc.sync.dma_start(out=w_k, in_=wk[:])

        xcs0 = []
        for g in range(NXC):
            xc = xp.tile([128, NK // NXC, QC], dt.bfloat16, tag=f"xc{g}",
                         name=f"xc{g}_0")
            nc.sync.dma_start(out=xc, in_=x[:, 0, g * 8:(g + 1) * 8])
            xcs0.append(xc)

        c_cos = singles.tile([DH, S], dt.float32, tag="cos")
        nc.sync.dma_start(out=c_cos, in_=cosT[:])
        c_sin = singles.tile([DH, S], dt.float32, tag="sin")
        nc.sync.dma_start(out=c_sin, in_=sinm[:])

        w_q = wqp.tile([128, NK, HPC, DH], dt.bfloat16, tag="wq")
        for h in range(HPC):
            nc.sync.dma_start(out=w_q[:, :, h], in_=wq[:, :, h])
        w_v = singles.tile([128, NK, DH], dt.bfloat16, tag="wv")
        nc.sync.dma_start(out=w_v, in_=wv[:])

        c_tri = singles.tile([128, 128], dt.bfloat16, tag="tri")
        nc.sync.dma_start(out=c_tri, in_=tri[:])
        c_id = singles.tile([128, 128], dt.bfloat16, tag="ident")
        nc.sync.dma_start(out=c_id, in_=ident[:])

        w_o = wop.tile([128, HPC, DIN], dt.bfloat16, tag="wo")
        for h in range(HPC):
            nc.sync.dma_start(out=w_o[:, h], in_=wo[:, h])

        c_bias = singles.tile([128, 1], dt.float32, tag="ebias")
        nc.vector.memset(c_bias, EXP_BIAS)

        # ---- long-lived activations ----
        qt = [qkv.tile([DH, S], dt.bfloat16, tag=f"qt{h}", name=f"qt{h}")
              for h in range(HPC)]
        kt = qkv.tile([DH, S], dt.bfloat16, tag="kt")
        vn = qkv.tile([128, NKT, DH], dt.bfloat16, tag="vn")   # V natural tiles
        ctxT = [qkv.tile([DH, S], dt.bfloat16, tag=f"ctx{h}", name=f"ctx{h}")
                for h in range(HPC)]

        def rope_from_psum(ps, dst_slice, s0):
            """dst = ps*cos + rot_half(ps)*sinm over s-columns [s0, s0+QC)."""
            t1 = tpool.tile([DH, QC], dt.float32, tag="t1", name="t1")
            nc.vector.tensor_mul(t1, ps, c_cos[:, s0:s0 + QC])
            t2 = tpool.tile([DH, QC], dt.float32, tag="t2", name="t2")
            nc.vector.tensor_mul(t2[0:64, :], ps[64:128, :], c_sin[0:64, s0:s0 + QC])
            nc.vector.tensor_mul(t2[64:128, :], ps[0:64, :], c_sin[64:128, s0:s0 + QC])
            nc.vector.tensor_add(dst_slice, t1, t2)

        def emit_proj(sq, xcs):
            """K/Q/V projections + RoPE + V transpose for quarter sq."""
            s0 = sq * QC
            psk = ps_proj.tile([DH, QC], dt.float32, tag="acc", name="psk")
            for k in range(NK):
                nc.tensor.matmul(psk, lhsT=w_k[:, k], rhs=xcs[k // 8][:, k % 8],
                                 start=(k == 0), stop=(k == NK - 1))
            rope_from_psum(psk, kt[:, s0:s0 + QC], s0)

            for h in range(HPC):
                psq = ps_proj.tile([DH, QC], dt.float32, tag="acc", name="psq")
                for k in range(NK):
                    nc.tensor.matmul(psq, lhsT=w_q[:, k, h],
                                     rhs=xcs[k // 8][:, k % 8],
                                     start=(k == 0), stop=(k == NK - 1))
                rope_from_psum(psq, qt[h][:, s0:s0 + QC], s0)

            psv = ps_proj.tile([DH, QC], dt.float32, tag="acc", name="psv")
            for k in range(NK):
                nc.tensor.matmul(psv, lhsT=w_v[:, k], rhs=xcs[k // 8][:, k % 8],
                                 start=(k == 0), stop=(k == NK - 1))
            vtmp = tpool.tile([DH, QC], dt.bfloat16, tag="vtmp", name="vtmp")
            nc.scalar.copy(vtmp, psv)
            for i in range(QC // 128):
                pvt = ps_sc.tile([128, 128], dt.bfloat16, tag="sc", name="pvt")
                nc.tensor.transpose(pvt, vtmp[:, i * 128:(i + 1) * 128], c_id)
                nc.vector.tensor_copy(vn[:, sq * 4 + i], pvt)

        def emit_attn_head(sq, h):
            """Causal attention for head h over quarter sq's queries.

            Scores are [key-tile, q] transposed; diagonal key-tiles are
            trimmed to the columns that aren't fully masked, and the
            128-wide triangle on the diagonal gets the 0/1 mask."""
            s0 = sq * QC
            njt = 4 * (sq + 1)
            sacc = spool.tile([128, QC], dt.bfloat16, tag="sacc", name="sacc")
            po = ps_po.tile([DH, QC], dt.float32, tag="po", name="po")
            for jt in range(njt):
                r = jt - (njt - 4)          # >=0 -> diagonal tile index
                c0 = 128 * r if r > 0 else 0
                psc = ps_sc.tile([128, QC], dt.float32, tag="sc", name="psc")
                nc.tensor.matmul(psc[:, c0:QC], lhsT=kt[:, jt * KT:(jt + 1) * KT],
                                 rhs=qt[h][:, s0 + c0:s0 + QC],
                                 start=True, stop=True)
                e = epool.tile([128, QC], dt.bfloat16, tag="e", name="e")
                nc.scalar.activation(out=e[:, c0:QC], in_=psc[:, c0:QC],
                                     func=mybir.ActivationFunctionType.Exp,
                                     bias=c_bias, scale=SCALE)
                if r >= 0:
                    nc.vector.tensor_mul(e[:, c0:c0 + 128], e[:, c0:c0 + 128],
                                         c_tri)
                if jt == 0:
                    nc.vector.tensor_copy(sacc, e[:, 0:QC])
                else:
                    nc.vector.tensor_add(sacc[:, c0:QC], sacc[:, c0:QC],
                                         e[:, c0:QC])
                nc.tensor.matmul(po[:, c0:QC], lhsT=vn[:, jt], rhs=e[:, c0:QC],
                                 start=(jt == 0), stop=(jt == njt - 1))
            # normalization: ctx = O * exp(-ln(colsum(E))); the colsum is an
            # all-partition reduce on gpsimd, ln/exp on ACT (same table set
            # as the score exps) -> no PE involvement, no table thrash.
            csb = npool.tile([128, QC], dt.float32, tag="nrm", name="csb")
            nc.gpsimd.partition_all_reduce(csb, sacc, channels=128,
                                           reduce_op=bass_isa.ReduceOp.add)
            lnt = npool.tile([128, QC], dt.float32, tag="nrm", name="lnt")
            nc.scalar.activation(out=lnt, in_=csb,
                                 func=mybir.ActivationFunctionType.Ln)
            rcs = npool.tile([128, QC], dt.float32, tag="nrm", name="rcs")
            nc.scalar.activation(out=rcs, in_=lnt,
                                 func=mybir.ActivationFunctionType.Exp,
                                 scale=-1.0)
            nc.vector.tensor_mul(ctxT[h][:, s0:s0 + QC], po, rcs)

        def emit_outproj_block(st):
            """out[st*128:(st+1)*128, :] = sum_h ctxT[h][:, st-block].T @ Wo[h]."""
            for half in range(2):
                ob = obp.tile([128, DIN // 2], dt.bfloat16, tag="ob", name="ob")
                for j in range(4):
                    oc = half * 4 + j
                    pso = ps_out.tile([128, 512], dt.float32, tag="pso", name="pso")
                    for h in range(HPC):
                        nc.tensor.matmul(pso,
                                         lhsT=ctxT[h][:, st * 128:(st + 1) * 128],
                                         rhs=w_o[:, h, oc * 512:(oc + 1) * 512],
                                         start=(h == 0), stop=(h == HPC - 1))
                    dst = ob[:, j * 512:(j + 1) * 512]
                    if oc % 2 == 0:
                        nc.scalar.copy(dst, pso)
                    else:
                        nc.vector.tensor_copy(dst, pso)
                nc.sync.dma_start(
                    out=out[st * 128:(st + 1) * 128,
                            half * (DIN // 2):(half + 1) * (DIN // 2)],
                    in_=ob)

        # ---- main pipeline ----
        xcs = xcs0
        for sq in range(NQ):
            emit_proj(sq, xcs)
            if sq + 1 < NQ:
                nxt = []
                for g in range(NXC):
                    xc = xp.tile([128, NK // NXC, QC], dt.bfloat16, tag=f"xc{g}",
                                 name=f"xc{g}_{sq + 1}")
                    nc.sync.dma_start(out=xc, in_=x[:, sq + 1, g * 8:(g + 1) * 8])
                    nxt.append(xc)
            for h in range(HPC):
                emit_attn_head(sq, h)
                if sq >= 1:
                    emit_outproj_block(4 * (sq - 1) + h)
            if sq + 1 < NQ:
                xcs = nxt
        for st in range(4 * (NQ - 1), 4 * NQ):
            emit_outproj_block(st)
    nc.finalize()
    return nc


def make_in_maps(input_tensor, cos, sin, Wq, Wk, Wv, Wo):
    """Host-side sharding + layout preparation. Returns list of 8 dicts."""
    x2 = np.ascontiguousarray(input_tensor.reshape(S, DIN))
    # x_host[p, sq, k, sc] = x2[512*sq+sc, 128*k+p]
    xt = x2.T.astype(BF16)                      # [DIN, S]
    x_host = np.ascontiguousarray(
        xt.reshape(NK, 128, NQ, QC).transpose(1, 2, 0, 3))

    cosT = np.ascontiguousarray(cos.T.astype(np.float32))
    sinm = np.ascontiguousarray(sin.T.astype(np.float32))
    sinm = sinm.copy()
    sinm[0:64, :] *= -1.0

    p_idx = np.arange(128)[:, None]
    c_idx = np.arange(128)[None, :]
    tri = (p_idx <= c_idx).astype(BF16)

    ident = np.eye(128, dtype=BF16)

    common = dict(x=x_host, cosT=cosT, sinm=sinm, tri=tri, ident=ident)

    in_maps = []
    for c in range(NCORES):
        wq_s = Wq[:, c * DPC:(c + 1) * DPC].astype(BF16)
        wq_host = np.ascontiguousarray(
            wq_s.reshape(NK, 128, HPC, DH).transpose(1, 0, 2, 3))
        wk_s = Wk[:, c * DH:(c + 1) * DH].astype(BF16)
        wk_host = np.ascontiguousarray(wk_s.reshape(NK, 128, DH).transpose(1, 0, 2))
        wv_s = Wv[:, c * DH:(c + 1) * DH].astype(BF16)
        wv_host = np.ascontiguousarray(wv_s.reshape(NK, 128, DH).transpose(1, 0, 2))
        wo_s = Wo[c * DPC:(c + 1) * DPC, :].astype(BF16)
        wo_host = np.ascontiguousarray(wo_s.reshape(HPC, 128, DIN).transpose(1, 0, 2))
        in_maps.append(dict(common, wq=wq_host, wk=wk_host, wv=wv_host, wo=wo_host))
    return in_maps


def _numpy_fallback(input_tensor, attention_mask, cos, sin, Wq, Wk, Wv, Wo):
    x = input_tensor.astype(np.float32)
    b, s, _ = x.shape
    q = (x @ Wq).reshape(b, s, H, DH).transpose(0, 2, 1, 3)
    k = (x @ Wk).reshape(b, s, KV, DH).transpose(0, 2, 1, 3)
    v = (x @ Wv).reshape(b, s, KV, DH).transpose(0, 2, 1, 3)

    def rope(t):
        t1, t2 = t[..., :64], t[..., 64:]
        rot = np.concatenate([-t2, t1], axis=-1)
        return t * cos[None, None] + rot * sin[None, None]

    q, k = rope(q), rope(k)
    k = np.repeat(k, G, axis=1)
    v = np.repeat(v, G, axis=1)
    sc = np.einsum('bhqd,bhkd->bhqk', q, k)
    sc = np.where(attention_mask, -np.inf, sc) / np.float32(np.sqrt(DH))
    sc = sc - sc.max(axis=-1, keepdims=True)
    w = np.exp(sc)
    w = w / w.sum(axis=-1, keepdims=True)
    ctx = np.einsum('bhqk,bhkd->bhqd', w, v)
    ctx = ctx.transpose(0, 2, 1, 3).reshape(b, s, H * DH)
    return (ctx @ Wo).astype(np.float32)


_NC_CACHE = {}


def kernel(input_tensor, attention_mask, cos, sin, Wq, Wk, Wv, Wo):
    mask = np.asarray(attention_mask).reshape(S, S)
    causal = np.array_equal(mask, np.triu(np.ones((S, S), bool), k=1))
    if not causal:
        return _numpy_fallback(np.asarray(input_tensor), np.asarray(attention_mask),
                               np.asarray(cos), np.asarray(sin),
                               np.asarray(Wq), np.asarray(Wk),
                               np.asarray(Wv), np.asarray(Wo))

    if "nc" not in _NC_CACHE:
        _NC_CACHE["nc"] = build_nc()
    nc = _NC_CACHE["nc"]

    in_maps = make_in_maps(np.asarray(input_tensor), np.asarray(cos),
                           np.asarray(sin), np.asarray(Wq), np.asarray(Wk),
                           np.asarray(Wv), np.asarray(Wo))
    res = run_bass_kernel_spmd(nc, in_maps, core_ids=list(range(NCORES)))
    acc = np.zeros((S, DIN), np.float32)
    for r in res.results:
        acc += np.asarray(r["out"], dtype=np.float32)
    return acc.reshape(1, S, DIN)


# revision 10
# speedup vs baseline: 1.1831x; 1.1219x over previous
"""GroupQueryAttention on 8 TRN2 NeuronCores.

Strategy: tensor-parallel over heads. H=32 query heads, KV=8 kv heads,
group size G=4 -> each core owns exactly 1 kv head and its 4 query heads.
Per core:
  - QKV projections from a replicated (pre-transposed, channels-major) input
  - RoPE on Q/K (rotate-half, done on DVE across partition halves)
  - attention with scores computed TRANSPOSED ([keys, q] layout) so the
    exp(scores) tiles feed the V-matmul directly as the moving operand;
    softmax normalization is deferred: O = V.E, then ctx = O * (1/colsum(E))
  - partial output ctx @ Wo_shard  (row-shard of Wo)
Host sums the 8 partial outputs (the "all-reduce" of the row-parallel Wo).

Perf structure (v3):
  - softmax normalization: colsum matmul (ones) -> DVE reciprocal_approx_fast
    -> rank-1 broadcast matmul -> ACT copy -> DVE scale. Short chain, no
    activation-table thrash (only Exp is ever used on ACT), no long DVE
    iterative divides
  - causal diagonal tiles trimmed: score/PV matmuls only cover the
    not-fully-masked query columns; a single 128x128 lower-tri mask
  - per-head projection emission (K, V, then Q_h right before head h's
    attention) so startup overlaps DMA, and out-projection of quarter q-1
    interleaves into quarter q's attention as PE filler
  - PSUM->SBUF drains split between ACT and DVE; output rows coalesced
    into [128, 2048] tiles before DMA
"""

import sys

sys.path.insert(0, "/opt/trn_rl_repo")

from contextlib import ExitStack

import numpy as np
import ml_dtypes

import concourse.bass as bass
import concourse.bacc as bacc
import concourse.tile as tile
from concourse import mybir
from concourse import bass_isa
from concourse.bass_utils import run_bass_kernel_spmd

BF16 = ml_dtypes.bfloat16

S = 2048          # sequence length
DIN = 4096        # model dim
H, KV, DH = 32, 8, 128
G = H // KV       # 4 query heads per kv head
NCORES = 8
HPC = H // NCORES     # 4 query heads per core
DPC = HPC * DH        # 512 = per-core q-projection width

NQ = 4            # s-quarters (chunks of 512 queries)
QC = S // NQ      # 512
KT = 128          # key tile (partition dim of transposed scores)
NKT = S // KT     # 16 key tiles
NK = DIN // 128   # 32 contraction tiles for projections
NXC = 4           # x chunks per quarter (k-groups of 8)
SCALE = 1.0 / float(np.sqrt(DH))
EXP_BIAS = -10.0  # constant shift inside exp; cancels in normalization


def build_nc():
    """Build the per-core Bass program (same program on all 8 cores; the
    per-core weight shards arrive via in_maps)."""
    nc = bacc.Bacc()
    dt = mybir.dt

    # ---- DRAM parameters (host-prepared layouts; all DMA-contiguous) ----
    # x[p, sq, k, sc] = x_orig[512*sq + sc, 128*k + p]   (channels-major)
    x = nc.declare_dram_parameter("x", [128, NQ, NK, QC], dt.bfloat16, isOutput=False)
    # wq[p, k, m, d] = Wq_shard[128*k + p, 128*m + d]
    wq = nc.declare_dram_parameter("wq", [128, NK, HPC, DH], dt.bfloat16, isOutput=False)
    # wk[p, k, d] = Wk_shard[128*k + p, d]
    wk = nc.declare_dram_parameter("wk", [128, NK, DH], dt.bfloat16, isOutput=False)
    wv = nc.declare_dram_parameter("wv", [128, NK, DH], dt.bfloat16, isOutput=False)
    # wo[p, h, n] = Wo_shard[128*h + p, n]
    wo = nc.declare_dram_parameter("wo", [128, HPC, DIN], dt.bfloat16, isOutput=False)
    # cosT[d, s] = cos[s, d]; sinm[d, s] = -sin[s, d] for d<64 else +sin[s, d]
    cosT = nc.declare_dram_parameter("cosT", [DH, S], dt.float32, isOutput=False)
    sinm = nc.declare_dram_parameter("sinm", [DH, S], dt.float32, isOutput=False)
    # tri[p, c] = 1.0 if p <= c else 0.0  (128x128 causal triangle)
    tri = nc.declare_dram_parameter("tri", [128, 128], dt.bfloat16, isOutput=False)
    ident = nc.declare_dram_parameter("ident", [128, 128], dt.bfloat16, isOutput=False)
    ones_col = nc.declare_dram_parameter("ones_col", [128, 1], dt.bfloat16, isOutput=False)
    ones_row = nc.declare_dram_parameter("ones_row", [1, 128], dt.float32, isOutput=False)
    out = nc.declare_dram_parameter("out", [S, DIN], dt.bfloat16, isOutput=True)

    with tile.TileContext(nc) as tc, ExitStack() as ctx:
        singles = ctx.enter_context(tc.tile_pool(name="singles", bufs=1))
        wqp = ctx.enter_context(tc.tile_pool(name="wqp", bufs=1))
        wop = ctx.enter_context(tc.tile_pool(name="wop", bufs=1))
        xp = ctx.enter_context(tc.tile_pool(name="xp", bufs=1))
        qkv = ctx.enter_context(tc.tile_pool(name="qkv", bufs=1))
        epool = ctx.enter_context(tc.tile_pool(name="epool", bufs=4))
        spool = ctx.enter_context(tc.tile_pool(name="spool", bufs=2))
        npool = ctx.enter_context(tc.tile_pool(name="npool", bufs=4))
        tpool = ctx.enter_context(tc.tile_pool(name="tpool", bufs=2))
        obp = ctx.enter_context(tc.tile_pool(name="obp", bufs=2))
        ps_proj = ctx.enter_context(tc.tile_pool(name="ps_proj", bufs=2, space="PSUM"))
        ps_po = ctx.enter_context(tc.tile_pool(name="ps_po", bufs=2, space="PSUM"))
        ps_sc = ctx.enter_context(tc.tile_pool(name="ps_sc", bufs=2, space="PSUM"))
        ps_out = ctx.enter_context(tc.tile_pool(name="ps_out", bufs=2, space="PSUM"))

        # ---- constants / weights resident in SBUF ----
        # DMA emission order == consumption order so the PE never stalls at
        # kernel start: wk + x chunks feed the K-projection, wv + wq_h0
        # right after, cos/sin for RoPE before quarter-0 attention; the
        # remaining q-head weights and wo trail in.
        w_k = singles.tile([128, NK, DH], dt.bfloat16, tag="wk")
        nc.sync.dma_start(out=w_k, in_=wk[:])

        xcs0 = []
        for g in range(NXC):
            xc = xp.tile([128, NK // NXC, QC], dt.bfloat16, tag=f"xc{g}",
                         name=f"xc{g}_0")
            nc.sync.dma_start(out=xc, in_=x[:, 0, g * 8:(g + 1) * 8])
            xcs0.append(xc)

        w_v = singles.tile([128, NK, DH], dt.bfloat16, tag="wv")
        nc.sync.dma_start(out=w_v, in_=wv[:])

        w_q = [wqp.tile([128, NK, DH], dt.bfloat16, tag=f"wq{h}", name=f"wq{h}")
               for h in range(HPC)]
        nc.sync.dma_start(out=w_q[0], in_=wq[:, :, 0])

        c_cos = singles.tile([DH, S], dt.float32, tag="cos")
        nc.sync.dma_start(out=c_cos, in_=cosT[:])
        c_sin = singles.tile([DH, S], dt.float32, tag="sin")
        nc.sync.dma_start(out=c_sin, in_=sinm[:])

        c_tri = singles.tile([128, 128], dt.bfloat16, tag="tri")
        nc.sync.dma_start(out=c_tri, in_=tri[:])
        c_id = singles.tile([128, 128], dt.bfloat16, tag="ident")
        nc.sync.dma_start(out=c_id, in_=ident[:])
        c_oc = singles.tile([128, 1], dt.bfloat16, tag="ones_col")
        nc.sync.dma_start(out=c_oc, in_=ones_col[:])
        c_or = singles.tile([1, 128], dt.float32, tag="ones_row")
        nc.sync.dma_start(out=c_or, in_=ones_row[:])

        for h in range(1, HPC):
            nc.sync.dma_start(out=w_q[h], in_=wq[:, :, h])

        w_o = wop.tile([128, HPC, DIN], dt.bfloat16, tag="wo")
        for h in range(HPC):
            nc.sync.dma_start(out=w_o[:, h], in_=wo[:, h])

        c_bias = singles.tile([128, 1], dt.float32, tag="ebias")
        nc.vector.memset(c_bias, EXP_BIAS)

        # ---- long-lived activations ----
        qt = [qkv.tile([DH, S], dt.bfloat16, tag=f"qt{h}", name=f"qt{h}")
              for h in range(HPC)]
        kt = qkv.tile([DH, S], dt.bfloat16, tag="kt")
        vn = qkv.tile([128, NKT, DH], dt.bfloat16, tag="vn")   # V natural tiles
        ctxT = [qkv.tile([DH, S], dt.bfloat16, tag=f"ctx{h}", name=f"ctx{h}")
                for h in range(HPC)]

        def rope_from_psum(ps, dst_slice, s0):
            """dst = ps*cos + rot_half(ps)*sinm over s-columns [s0, s0+QC)."""
            t1 = tpool.tile([DH, QC], dt.float32, tag="t1", name="t1")
            nc.vector.tensor_mul(t1, ps, c_cos[:, s0:s0 + QC])
            t2 = tpool.tile([DH, QC], dt.float32, tag="t2", name="t2")
            nc.vector.tensor_mul(t2[0:64, :], ps[64:128, :], c_sin[0:64, s0:s0 + QC])
            nc.vector.tensor_mul(t2[64:128, :], ps[0:64, :], c_sin[64:128, s0:s0 + QC])
            nc.vector.tensor_add(dst_slice, t1, t2)

        def emit_kv(sq, xcs):
            """K and V projections + RoPE(K) + V transpose for quarter sq."""
            s0 = sq * QC
            psk = ps_proj.tile([DH, QC], dt.float32, tag="acc", name="psk")
            for k in range(NK):
                nc.tensor.matmul(psk, lhsT=w_k[:, k], rhs=xcs[k // 8][:, k % 8],
                                 start=(k == 0), stop=(k == NK - 1))
            rope_from_psum(psk, kt[:, s0:s0 + QC], s0)

            psv = ps_proj.tile([DH, QC], dt.float32, tag="acc", name="psv")
            for k in range(NK):
                nc.tensor.matmul(psv, lhsT=w_v[:, k], rhs=xcs[k // 8][:, k % 8],
                                 start=(k == 0), stop=(k == NK - 1))
            vtmp = tpool.tile([DH, QC], dt.bfloat16, tag="vtmp", name="vtmp")
            nc.scalar.copy(vtmp, psv)
            for i in range(QC // 128):
                pvt = ps_sc.tile([128, 128], dt.bfloat16, tag="sc", name="pvt")
                nc.tensor.transpose(pvt, vtmp[:, i * 128:(i + 1) * 128], c_id)
                nc.vector.tensor_copy(vn[:, sq * 4 + i], pvt)

        def emit_q(sq, h, xcs):
            """Q projection + RoPE for head h, quarter sq."""
            s0 = sq * QC
            psq = ps_proj.tile([DH, QC], dt.float32, tag="acc", name="psq")
            for k in range(NK):
                nc.tensor.matmul(psq, lhsT=w_q[h][:, k],
                                 rhs=xcs[k // 8][:, k % 8],
                                 start=(k == 0), stop=(k == NK - 1))
            rope_from_psum(psq, qt[h][:, s0:s0 + QC], s0)

        def emit_attn_head(sq, h):
            """Causal attention for head h over quarter sq's queries.

            Scores are [key-tile, q] transposed; diagonal key-tiles are
            trimmed to the columns that aren't fully masked, and the
            128-wide triangle on the diagonal gets the 0/1 mask."""
            s0 = sq * QC
            njt = 4 * (sq + 1)
            sacc = spool.tile([128, QC], dt.bfloat16, tag="sacc", name="sacc")
            po = ps_po.tile([DH, QC], dt.float32, tag="po", name="po")
            for jt in range(njt):
                r = jt - (njt - 4)          # >=0 -> diagonal tile index
                c0 = 128 * r if r > 0 else 0
                psc = ps_sc.tile([128, QC], dt.float32, tag="sc", name="psc")
                nc.tensor.matmul(psc[:, c0:QC], lhsT=kt[:, jt * KT:(jt + 1) * KT],
                                 rhs=qt[h][:, s0 + c0:s0 + QC],
                                 start=True, stop=True)
                e = epool.tile([128, QC], dt.bfloat16, tag="e", name="e")
                nc.scalar.activation(out=e[:, c0:QC], in_=psc[:, c0:QC],
                                     func=mybir.ActivationFunctionType.Exp,
                                     bias=c_bias, scale=SCALE)
                if r >= 0:
                    nc.vector.tensor_mul(e[:, c0:c0 + 128], e[:, c0:c0 + 128],
                                         c_tri)
                if jt == 0:
                    nc.vector.tensor_copy(sacc, e[:, 0:QC])
                else:
                    nc.vector.tensor_add(sacc[:, c0:QC], sacc[:, c0:QC],
                                         e[:, c0:QC])
                nc.tensor.matmul(po[:, c0:QC], lhsT=vn[:, jt], rhs=e[:, c0:QC],
                                 start=(jt == 0), stop=(jt == njt - 1))
            # normalization: ctx = O * (1/colsum(E)). colsum via a ones
            # matmul, fast approximate reciprocal on DVE (single custom op,
            # ~51 ULP), rank-1 matmul broadcast, ACT drain, DVE scale.
            # Short chain, nothing expensive on any engine.
            pcs = ps_out.tile([1, QC], dt.float32, tag="pso", name="pcs")
            nc.tensor.matmul(pcs, lhsT=c_oc, rhs=sacc, start=True, stop=True)
            rec = npool.tile([1, QC], dt.float32, tag="rec", name="rec")
            nc.vector.reciprocal_approx_fast(out=rec, in_=pcs)
            prb = ps_out.tile([128, QC], dt.float32, tag="pso", name="prb")
            nc.tensor.matmul(prb, lhsT=c_or, rhs=rec, start=True, stop=True)
            rcs = npool.tile([128, QC], dt.float32, tag="nrm", name="rcs")
            nc.scalar.copy(rcs, prb)
            nc.vector.tensor_mul(ctxT[h][:, s0:s0 + QC], po, rcs)

        def emit_outproj_block(st):
            """out[st*128:(st+1)*128, :] = sum_h ctxT[h][:, st-block].T @ Wo[h]."""
            for quad in range(4):
                ob = obp.tile([128, DIN // 4], dt.bfloat16, tag="ob", name="ob")
                for j in range(2):
                    oc = quad * 2 + j
                    pso = ps_out.tile([128, 512], dt.float32, tag="pso", name="pso")
                    for h in range(HPC):
                        nc.tensor.matmul(pso,
                                         lhsT=ctxT[h][:, st * 128:(st + 1) * 128],
                                         rhs=w_o[:, h, oc * 512:(oc + 1) * 512],
                                         start=(h == 0), stop=(h == HPC - 1))
                    dst = ob[:, j * 512:(j + 1) * 512]
                    if oc % 2 == 0:
                        nc.scalar.copy(dst, pso)
                    else:
                        nc.vector.tensor_copy(dst, pso)
                nc.sync.dma_start(
                    out=out[st * 128:(st + 1) * 128,
                            quad * (DIN // 4):(quad + 1) * (DIN // 4)],
                    in_=ob)

        # ---- main pipeline ----
        # Per quarter: K/V projections, then per head [Q_h, attention_h,
        # out-proj block of the previous quarter]. The out-proj matmuls are
        # independent PE filler while attention waits on exp results; the
        # x chunks for the next quarter prefetch once the last Q reads them.
        xcs = xcs0
        for sq in range(NQ):
            emit_kv(sq, xcs)
            for h in range(HPC):
                emit_q(sq, h, xcs)
                if h == HPC - 1 and sq + 1 < NQ:
                    nxt = []
                    for g in range(NXC):
                        xc = xp.tile([128, NK // NXC, QC], dt.bfloat16,
                                     tag=f"xc{g}", name=f"xc{g}_{sq + 1}")
                        nc.sync.dma_start(out=xc,
                                          in_=x[:, sq + 1, g * 8:(g + 1) * 8])
                        nxt.append(xc)
                emit_attn_head(sq, h)
                if sq >= 1:
                    emit_outproj_block(4 * (sq - 1) + h)
            if sq + 1 < NQ:
                xcs = nxt
        for st in range(4 * (NQ - 1), 4 * NQ):
            emit_outproj_block(st)
    nc.finalize()
    return nc


def make_in_maps(input_tensor, cos, sin, Wq, Wk, Wv, Wo):
    """Host-side sharding + layout preparation. Returns list of 8 dicts."""
    x2 = np.ascontiguousarray(input_tensor.reshape(S, DIN))
    # x_host[p, sq, k, sc] = x2[512*sq+sc, 128*k+p]
    xt = x2.T.astype(BF16)                      # [DIN, S]
    x_host = np.ascontiguousarray(
        xt.reshape(NK, 128, NQ, QC).transpose(1, 2, 0, 3))

    cosT = np.ascontiguousarray(cos.T.astype(np.float32))
    sinm = np.ascontiguousarray(sin.T.astype(np.float32))
    sinm = sinm.copy()
    sinm[0:64, :] *= -1.0

    p_idx = np.arange(128)[:, None]
    c_idx = np.arange(128)[None, :]
    tri = (p_idx <= c_idx).astype(BF16)

    ident = np.eye(128, dtype=BF16)
    ones_col = np.ones((128, 1), dtype=BF16)
    ones_row = np.ones((1, 128), dtype=np.float32)

    common = dict(x=x_host, cosT=cosT, sinm=sinm, tri=tri, ident=ident,
                  ones_col=ones_col, ones_row=ones_row)

    in_maps = []
    for c in range(NCORES):
        wq_s = Wq[:, c * DPC:(c + 1) * DPC].astype(BF16)
        wq_host = np.ascontiguousarray(
            wq_s.reshape(NK, 128, HPC, DH).transpose(1, 0, 2, 3))
        wk_s = Wk[:, c * DH:(c + 1) * DH].astype(BF16)
        wk_host = np.ascontiguousarray(wk_s.reshape(NK, 128, DH).transpose(1, 0, 2))
        wv_s = Wv[:, c * DH:(c + 1) * DH].astype(BF16)
        wv_host = np.ascontiguousarray(wv_s.reshape(NK, 128, DH).transpose(1, 0, 2))
        wo_s = Wo[c * DPC:(c + 1) * DPC, :].astype(BF16)
        wo_host = np.ascontiguousarray(wo_s.reshape(HPC, 128, DIN).transpose(1, 0, 2))
        in_maps.append(dict(common, wq=wq_host, wk=wk_host, wv=wv_host, wo=wo_host))
    return in_maps


def _numpy_fallback(input_tensor, attention_mask, cos, sin, Wq, Wk, Wv, Wo):
    x = input_tensor.astype(np.float32)
    b, s, _ = x.shape
    q = (x @ Wq).reshape(b, s, H, DH).transpose(0, 2, 1, 3)
    k = (x @ Wk).reshape(b, s, KV, DH).transpose(0, 2, 1, 3)
    v = (x @ Wv).reshape(b, s, KV, DH).transpose(0, 2, 1, 3)

    def rope(t):
        t1, t2 = t[..., :64], t[..., 64:]
        rot = np.concatenate([-t2, t1], axis=-1)
        return t * cos[None, None] + rot * sin[None, None]

    q, k = rope(q), rope(k)
    k = np.repeat(k, G, axis=1)
    v = np.repeat(v, G, axis=1)
    sc = np.einsum('bhqd,bhkd->bhqk', q, k)
    sc = np.where(attention_mask, -np.inf, sc) / np.float32(np.sqrt(DH))
    sc = sc - sc.max(axis=-1, keepdims=True)
    w = np.exp(sc)
    w = w / w.sum(axis=-1, keepdims=True)
    ctx = np.einsum('bhqk,bhkd->bhqd', w, v)
    ctx = ctx.transpose(0, 2, 1, 3).reshape(b, s, H * DH)
    return (ctx @ Wo).astype(np.float32)


_NC_CACHE = {}


def kernel(input_tensor, attention_mask, cos, sin, Wq, Wk, Wv, Wo):
    mask = np.asarray(attention_mask).reshape(S, S)
    causal = np.array_equal(mask, np.triu(np.ones((S, S), bool), k=1))
    if not causal:
        return _numpy_fallback(np.asarray(input_tensor), np.asarray(attention_mask),
                               np.asarray(cos), np.asarray(sin),
                               np.asarray(Wq), np.asarray(Wk),
                               np.asarray(Wv), np.asarray(Wo))

    if "nc" not in _NC_CACHE:
        _NC_CACHE["nc"] = build_nc()
    nc = _NC_CACHE["nc"]

    in_maps = make_in_maps(np.asarray(input_tensor), np.asarray(cos),
                           np.asarray(sin), np.asarray(Wq), np.asarray(Wk),
                           np.asarray(Wv), np.asarray(Wo))
    res = run_bass_kernel_spmd(nc, in_maps, core_ids=list(range(NCORES)))
    acc = np.zeros((S, DIN), np.float32)
    for r in res.results:
        acc += np.asarray(r["out"], dtype=np.float32)
    return acc.reshape(1, S, DIN)


# revision 14
# speedup vs baseline: 1.1925x; 1.0079x over previous
"""GroupQueryAttention on 8 TRN2 NeuronCores.

Strategy: tensor-parallel over heads. H=32 query heads, KV=8 kv heads,
group size G=4 -> each core owns exactly 1 kv head and its 4 query heads.
Per core:
  - QKV projections from a replicated (pre-transposed, channels-major) input
  - RoPE on Q/K (rotate-half, done on DVE across partition halves)
  - attention with scores computed TRANSPOSED ([keys, q] layout) so the
    exp(scores) tiles feed the V-matmul directly as the moving operand;
    softmax normalization is deferred: O = V.E, then ctx = O * (1/colsum(E))
  - partial output ctx @ Wo_shard  (row-shard of Wo)
Host sums the 8 partial outputs (the "all-reduce" of the row-parallel Wo).

Perf structure (v3):
  - softmax normalization: colsum matmul (ones) -> DVE reciprocal_approx_fast
    -> rank-1 broadcast matmul -> ACT copy -> DVE scale. Short chain, no
    activation-table thrash (only Exp is ever used on ACT), no long DVE
    iterative divides
  - causal diagonal tiles trimmed: score/PV matmuls only cover the
    not-fully-masked query columns; a single 128x128 lower-tri mask
  - per-head projection emission (K, V, then Q_h right before head h's
    attention) so startup overlaps DMA, and out-projection of quarter q-1
    interleaves into quarter q's attention as PE filler
  - PSUM->SBUF drains split between ACT and DVE; output rows coalesced
    into [128, 2048] tiles before DMA
"""

import sys

sys.path.insert(0, "/opt/trn_rl_repo")

from contextlib import ExitStack

import numpy as np
import ml_dtypes

import concourse.bass as bass
import concourse.bacc as bacc
import concourse.tile as tile
from concourse import mybir
from concourse import bass_isa
from concourse.bass_utils import run_bass_kernel_spmd

BF16 = ml_dtypes.bfloat16

S = 2048          # sequence length
DIN = 4096        # model dim
H, KV, DH = 32, 8, 128
G = H // KV       # 4 query heads per kv head
NCORES = 8
HPC = H // NCORES     # 4 query heads per core
DPC = HPC * DH        # 512 = per-core q-projection width

NQ = 4            # s-quarters (chunks of 512 queries)
QC = S // NQ      # 512
KT = 128          # key tile (partition dim of transposed scores)
NKT = S // KT     # 16 key tiles
NK = DIN // 128   # 32 contraction tiles for projections
NXC = 4           # x chunks per quarter (k-groups of 8)
SCALE = 1.0 / float(np.sqrt(DH))
EXP_BIAS = -10.0  # constant shift inside exp; cancels in normalization


def build_nc():
    """Build the per-core Bass program (same program on all 8 cores; the
    per-core weight shards arrive via in_maps)."""
    nc = bacc.Bacc()
    dt = mybir.dt

    # ---- DRAM parameters (host-prepared layouts; all DMA-contiguous) ----
    # x[p, sq, k, sc] = x_orig[512*sq + sc, 128*k + p]   (channels-major)
    x = nc.declare_dram_parameter("x", [128, NQ, NK, QC], dt.bfloat16, isOutput=False)
    # wq[p, k, m, d] = Wq_shard[128*k + p, 128*m + d]
    wq = nc.declare_dram_parameter("wq", [128, NK, HPC, DH], dt.bfloat16, isOutput=False)
    # wk[p, k, d] = Wk_shard[128*k + p, d]
    wk = nc.declare_dram_parameter("wk", [128, NK, DH], dt.bfloat16, isOutput=False)
    wv = nc.declare_dram_parameter("wv", [128, NK, DH], dt.bfloat16, isOutput=False)
    # wo[p, h, n] = Wo_shard[128*h + p, n]
    wo = nc.declare_dram_parameter("wo", [128, HPC, DIN], dt.bfloat16, isOutput=False)
    # cosT[d, s] = cos[s, d]; sinm[d, s] = -sin[s, d] for d<64 else +sin[s, d]
    cosT = nc.declare_dram_parameter("cosT", [DH, S], dt.float32, isOutput=False)
    sinm = nc.declare_dram_parameter("sinm", [DH, S], dt.float32, isOutput=False)
    # tri[p, c] = 1.0 if p <= c else 0.0  (128x128 causal triangle)
    tri = nc.declare_dram_parameter("tri", [128, 128], dt.bfloat16, isOutput=False)
    ident = nc.declare_dram_parameter("ident", [128, 128], dt.bfloat16, isOutput=False)
    ones_col = nc.declare_dram_parameter("ones_col", [128, 1], dt.bfloat16, isOutput=False)
    ones_row = nc.declare_dram_parameter("ones_row", [1, 128], dt.float32, isOutput=False)
    out = nc.declare_dram_parameter("out", [S, DIN], dt.bfloat16, isOutput=True)

    with tile.TileContext(nc) as tc, ExitStack() as ctx:
        singles = ctx.enter_context(tc.tile_pool(name="singles", bufs=1))
        wqp = ctx.enter_context(tc.tile_pool(name="wqp", bufs=1))
        wop = ctx.enter_context(tc.tile_pool(name="wop", bufs=1))
        xp = ctx.enter_context(tc.tile_pool(name="xp", bufs=1))
        qkv = ctx.enter_context(tc.tile_pool(name="qkv", bufs=1))
        epool = ctx.enter_context(tc.tile_pool(name="epool", bufs=4))
        spool = ctx.enter_context(tc.tile_pool(name="spool", bufs=2))
        npool = ctx.enter_context(tc.tile_pool(name="npool", bufs=4))
        tpool = ctx.enter_context(tc.tile_pool(name="tpool", bufs=2))
        obp = ctx.enter_context(tc.tile_pool(name="obp", bufs=3))
        ps_proj = ctx.enter_context(tc.tile_pool(name="ps_proj", bufs=2, space="PSUM"))
        ps_po = ctx.enter_context(tc.tile_pool(name="ps_po", bufs=2, space="PSUM"))
        ps_sc = ctx.enter_context(tc.tile_pool(name="ps_sc", bufs=2, space="PSUM"))
        ps_out = ctx.enter_context(tc.tile_pool(name="ps_out", bufs=2, space="PSUM"))

        # ---- constants / weights resident in SBUF ----
        # DMA emission order == consumption order so the PE never stalls at
        # kernel start: wk + x chunks feed the K-projection, wv + wq_h0
        # right after, cos/sin for RoPE before quarter-0 attention; the
        # remaining q-head weights and wo trail in.
        w_k = singles.tile([128, NK, DH], dt.bfloat16, tag="wk")
        nc.sync.dma_start(out=w_k[:, 0:8], in_=wk[:, 0:8])

        xcs0 = []
        for g in range(NXC):
            xc = xp.tile([128, NK // NXC, QC], dt.bfloat16, tag=f"xc{g}",
                         name=f"xc{g}_0")
            nc.sync.dma_start(out=xc, in_=x[:, 0, g * 8:(g + 1) * 8])
            xcs0.append(xc)
            if g < NXC - 1:
                nc.sync.dma_start(out=w_k[:, 8 * (g + 1):8 * (g + 2)],
                                  in_=wk[:, 8 * (g + 1):8 * (g + 2)])

        w_v = singles.tile([128, NK, DH], dt.bfloat16, tag="wv")
        nc.sync.dma_start(out=w_v, in_=wv[:])

        w_q = [wqp.tile([128, NK, DH], dt.bfloat16, tag=f"wq{h}", name=f"wq{h}")
               for h in range(HPC)]
        nc.sync.dma_start(out=w_q[0], in_=wq[:, :, 0])

        c_cos = singles.tile([DH, S], dt.float32, tag="cos")
        nc.sync.dma_start(out=c_cos, in_=cosT[:])
        c_sin = singles.tile([DH, S], dt.float32, tag="sin")
        nc.sync.dma_start(out=c_sin, in_=sinm[:])

        c_tri = singles.tile([128, 128], dt.bfloat16, tag="tri")
        nc.sync.dma_start(out=c_tri, in_=tri[:])
        c_id = singles.tile([128, 128], dt.bfloat16, tag="ident")
        nc.sync.dma_start(out=c_id, in_=ident[:])
        c_oc = singles.tile([128, 1], dt.bfloat16, tag="ones_col")
        nc.sync.dma_start(out=c_oc, in_=ones_col[:])
        c_or = singles.tile([1, 128], dt.float32, tag="ones_row")
        nc.sync.dma_start(out=c_or, in_=ones_row[:])

        for h in range(1, HPC):
            nc.sync.dma_start(out=w_q[h], in_=wq[:, :, h])

        w_o = wop.tile([128, HPC, DIN], dt.bfloat16, tag="wo")
        for h in range(HPC):
            nc.sync.dma_start(out=w_o[:, h], in_=wo[:, h])

        c_bias = singles.tile([128, 1], dt.float32, tag="ebias")
        nc.vector.memset(c_bias, EXP_BIAS)

        # ---- long-lived activations ----
        qt = [qkv.tile([DH, S], dt.bfloat16, tag=f"qt{h}", name=f"qt{h}")
              for h in range(HPC)]
        kt = qkv.tile([DH, S], dt.bfloat16, tag="kt")
        vn = qkv.tile([128, NKT, DH], dt.bfloat16, tag="vn")   # V natural tiles
        ctxT = [qkv.tile([DH, S], dt.bfloat16, tag=f"ctx{h}", name=f"ctx{h}")
                for h in range(HPC)]

        def rope_from_psum(ps, dst_slice, s0):
            """dst = ps*cos + rot_half(ps)*sinm over s-columns [s0, s0+QC)."""
            t1 = tpool.tile([DH, QC], dt.float32, tag="t1", name="t1")
            nc.vector.tensor_mul(t1, ps, c_cos[:, s0:s0 + QC])
            t2 = tpool.tile([DH, QC], dt.float32, tag="t2", name="t2")
            nc.vector.tensor_mul(t2[0:64, :], ps[64:128, :], c_sin[0:64, s0:s0 + QC])
            nc.vector.tensor_mul(t2[64:128, :], ps[0:64, :], c_sin[64:128, s0:s0 + QC])
            nc.vector.tensor_add(dst_slice, t1, t2)

        def emit_kv(sq, xcs):
            """K and V projections + RoPE(K) + V transpose for quarter sq."""
            s0 = sq * QC
            psk = ps_proj.tile([DH, QC], dt.float32, tag="acc", name="psk")
            for k in range(NK):
                nc.tensor.matmul(psk, lhsT=w_k[:, k], rhs=xcs[k // 8][:, k % 8],
                                 start=(k == 0), stop=(k == NK - 1))
            rope_from_psum(psk, kt[:, s0:s0 + QC], s0)

            psv = ps_proj.tile([DH, QC], dt.float32, tag="acc", name="psv")
            for k in range(NK):
                nc.tensor.matmul(psv, lhsT=w_v[:, k], rhs=xcs[k // 8][:, k % 8],
                                 start=(k == 0), stop=(k == NK - 1))
            vtmp = tpool.tile([DH, QC], dt.bfloat16, tag="vtmp", name="vtmp")
            nc.scalar.copy(vtmp, psv)
            for i in range(QC // 128):
                pvt = ps_sc.tile([128, 128], dt.bfloat16, tag="sc", name="pvt")
                nc.tensor.transpose(pvt, vtmp[:, i * 128:(i + 1) * 128], c_id)
                nc.vector.tensor_copy(vn[:, sq * 4 + i], pvt)

        def emit_q(sq, h, xcs):
            """Q projection + RoPE for head h, quarter sq.

            The x-chunk read order is rotated per head so that the last Q
            stream (h3) reads chunk 0 first -> next quarter's chunk-0
            prefetch DMA (WAR on these reads) can start that much sooner,
            staggered in the same order the next quarter consumes them."""
            s0 = sq * QC
            psq = ps_proj.tile([DH, QC], dt.float32, tag="acc", name="psq")
            korder = [(8 * (h + 1 + g) % NK) + kk
                      for g in range(NXC) for kk in range(8)]
            for i, k in enumerate(korder):
                nc.tensor.matmul(psq, lhsT=w_q[h][:, k],
                                 rhs=xcs[k // 8][:, k % 8],
                                 start=(i == 0), stop=(i == NK - 1))
            rope_from_psum(psq, qt[h][:, s0:s0 + QC], s0)

        def emit_attn_head(sq, h):
            """Causal attention for head h over quarter sq's queries.

            Scores are [key-tile, q] transposed; diagonal key-tiles are
            trimmed to the columns that aren't fully masked, and the
            128-wide triangle on the diagonal gets the 0/1 mask."""
            s0 = sq * QC
            njt = 4 * (sq + 1)
            sacc = spool.tile([128, QC], dt.bfloat16, tag="sacc", name="sacc")
            po = ps_po.tile([DH, QC], dt.float32, tag="po", name="po")
            for jt in range(njt):
                r = jt - (njt - 4)          # >=0 -> diagonal tile index
                c0 = 128 * r if r > 0 else 0
                psc = ps_sc.tile([128, QC], dt.float32, tag="sc", name="psc")
                nc.tensor.matmul(psc[:, c0:QC], lhsT=kt[:, jt * KT:(jt + 1) * KT],
                                 rhs=qt[h][:, s0 + c0:s0 + QC],
                                 start=True, stop=True)
                e = epool.tile([128, QC], dt.bfloat16, tag="e", name="e")
                nc.scalar.activation(out=e[:, c0:QC], in_=psc[:, c0:QC],
                                     func=mybir.ActivationFunctionType.Exp,
                                     bias=c_bias, scale=SCALE)
                if r >= 0:
                    nc.vector.tensor_mul(e[:, c0:c0 + 128], e[:, c0:c0 + 128],
                                         c_tri)
                if jt == 0:
                    nc.vector.tensor_copy(sacc, e[:, 0:QC])
                else:
                    nc.vector.tensor_add(sacc[:, c0:QC], sacc[:, c0:QC],
                                         e[:, c0:QC])
                nc.tensor.matmul(po[:, c0:QC], lhsT=vn[:, jt], rhs=e[:, c0:QC],
                                 start=(jt == 0), stop=(jt == njt - 1))
            # normalization: ctx = O * (1/colsum(E)). colsum via a ones
            # matmul, fast approximate reciprocal on DVE (single custom op,
            # ~51 ULP), rank-1 matmul broadcast, ACT drain, DVE scale.
            # Short chain, nothing expensive on any engine.
            pcs = ps_out.tile([1, QC], dt.float32, tag="pso", name="pcs")
            nc.tensor.matmul(pcs, lhsT=c_oc, rhs=sacc, start=True, stop=True)
            rec = npool.tile([1, QC], dt.float32, tag="rec", name="rec")
            nc.vector.reciprocal_approx_fast(out=rec, in_=pcs)
            prb = ps_out.tile([128, QC], dt.float32, tag="pso", name="prb")
            nc.tensor.matmul(prb, lhsT=c_or, rhs=rec, start=True, stop=True)
            rcs = npool.tile([128, QC], dt.float32, tag="nrm", name="rcs")
            nc.scalar.copy(rcs, prb)
            nc.vector.tensor_mul(ctxT[h][:, s0:s0 + QC], po, rcs)

        def emit_outproj_block(st):
            """out[st*128:(st+1)*128, :] = sum_h ctxT[h][:, st-block].T @ Wo[h]."""
            for quad in range(4):
                ob = obp.tile([128, DIN // 4], dt.bfloat16, tag="ob", name="ob")
                for j in range(2):
                    oc = quad * 2 + j
                    pso = ps_out.tile([128, 512], dt.float32, tag="pso", name="pso")
                    for h in range(HPC):
                        nc.tensor.matmul(pso,
                                         lhsT=ctxT[h][:, st * 128:(st + 1) * 128],
                                         rhs=w_o[:, h, oc * 512:(oc + 1) * 512],
                                         start=(h == 0), stop=(h == HPC - 1))
                    dst = ob[:, j * 512:(j + 1) * 512]
                    if oc % 2 == 0:
                        nc.scalar.copy(dst, pso)
                    else:
                        nc.vector.tensor_copy(dst, pso)
                nc.sync.dma_start(
                    out=out[st * 128:(st + 1) * 128,
                            quad * (DIN // 4):(quad + 1) * (DIN // 4)],
                    in_=ob)

        # ---- main pipeline ----
        # Per quarter: K/V projections, then per head [Q_h, attention_h,
        # out-proj block of the previous quarter]. The out-proj matmuls are
        # independent PE filler while attention waits on exp results; the
        # x chunks for the next quarter prefetch once the last Q reads them.
        xcs = xcs0
        for sq in range(NQ):
            emit_kv(sq, xcs)
            for h in range(HPC):
                emit_q(sq, h, xcs)
                if h == HPC - 1 and sq + 1 < NQ:
                    nxt = []
                    for g in range(NXC):
                        xc = xp.tile([128, NK // NXC, QC], dt.bfloat16,
                                     tag=f"xc{g}", name=f"xc{g}_{sq + 1}")
                        nc.sync.dma_start(out=xc,
                                          in_=x[:, sq + 1, g * 8:(g + 1) * 8])
                        nxt.append(xc)
                emit_attn_head(sq, h)
                # out-proj blocks of the previous quarter fill PE bubbles at
                # heads 0-2; block 4*sq at the quarter boundary (right after
                # head 3) covers the gap while the next quarter's x arrives.
                if sq >= 1 and h < HPC - 1:
                    emit_outproj_block(4 * (sq - 1) + h + 1)
            emit_outproj_block(4 * sq)
            if sq + 1 < NQ:
                xcs = nxt
        for st in range(4 * (NQ - 1) + 1, 4 * NQ):
            emit_outproj_block(st)
    nc.finalize()
    return nc


def make_in_maps(input_tensor, cos, sin, Wq, Wk, Wv, Wo):
    """Host-side sharding + layout preparation. Returns list of 8 dicts."""
    x2 = np.ascontiguousarray(input_tensor.reshape(S, DIN))
    # x_host[p, sq, k, sc] = x2[512*sq+sc, 128*k+p]
    xt = x2.T.astype(BF16)                      # [DIN, S]
    x_host = np.ascontiguousarray(
        xt.reshape(NK, 128, NQ, QC).transpose(1, 2, 0, 3))

    cosT = np.ascontiguousarray(cos.T.astype(np.float32))
    sinm = np.ascontiguousarray(sin.T.astype(np.float32))
    sinm = sinm.copy()
    sinm[0:64, :] *= -1.0

    p_idx = np.arange(128)[:, None]
    c_idx = np.arange(128)[None, :]
    tri = (p_idx <= c_idx).astype(BF16)

    ident = np.eye(128, dtype=BF16)
    ones_col = np.ones((128, 1), dtype=BF16)
    ones_row = np.ones((1, 128), dtype=np.float32)

    common = dict(x=x_host, cosT=cosT, sinm=sinm, tri=tri, ident=ident,
                  ones_col=ones_col, ones_row=ones_row)

    in_maps = []
    for c in range(NCORES):
        wq_s = Wq[:, c * DPC:(c + 1) * DPC].astype(BF16)
        wq_host = np.ascontiguousarray(
            wq_s.reshape(NK, 128, HPC, DH).transpose(1, 0, 2, 3))
        wk_s = Wk[:, c * DH:(c + 1) * DH].astype(BF16)
        wk_host = np.ascontiguousarray(wk_s.reshape(NK, 128, DH).transpose(1, 0, 2))
        wv_s = Wv[:, c * DH:(c + 1) * DH].astype(BF16)
        wv_host = np.ascontiguousarray(wv_s.reshape(NK, 128, DH).transpose(1, 0, 2))
        wo_s = Wo[c * DPC:(c + 1) * DPC, :].astype(BF16)
        wo_host = np.ascontiguousarray(wo_s.reshape(HPC, 128, DIN).transpose(1, 0, 2))
        in_maps.append(dict(common, wq=wq_host, wk=wk_host, wv=wv_host, wo=wo_host))
    return in_maps


def _numpy_fallback(input_tensor, attention_mask, cos, sin, Wq, Wk, Wv, Wo):
    x = input_tensor.astype(np.float32)
    b, s, _ = x.shape
    q = (x @ Wq).reshape(b, s, H, DH).transpose(0, 2, 1, 3)
    k = (x @ Wk).reshape(b, s, KV, DH).transpose(0, 2, 1, 3)
    v = (x @ Wv).reshape(b, s, KV, DH).transpose(0, 2, 1, 3)

    def rope(t):
        t1, t2 = t[..., :64], t[..., 64:]
        rot = np.concatenate([-t2, t1], axis=-1)
        return t * cos[None, None] + rot * sin[None, None]

    q, k = rope(q), rope(k)
    k = np.repeat(k, G, axis=1)
    v = np.repeat(v, G, axis=1)
    sc = np.einsum('bhqd,bhkd->bhqk', q, k)
    sc = np.where(attention_mask, -np.inf, sc) / np.float32(np.sqrt(DH))
    sc = sc - sc.max(axis=-1, keepdims=True)
    w = np.exp(sc)
    w = w / w.sum(axis=-1, keepdims=True)
    ctx = np.einsum('bhqk,bhkd->bhqd', w, v)
    ctx = ctx.transpose(0, 2, 1, 3).reshape(b, s, H * DH)
    return (ctx @ Wo).astype(np.float32)


_NC_CACHE = {}


def kernel(input_tensor, attention_mask, cos, sin, Wq, Wk, Wv, Wo):
    mask = np.asarray(attention_mask).reshape(S, S)
    causal = np.array_equal(mask, np.triu(np.ones((S, S), bool), k=1))
    if not causal:
        return _numpy_fallback(np.asarray(input_tensor), np.asarray(attention_mask),
                               np.asarray(cos), np.asarray(sin),
                               np.asarray(Wq), np.asarray(Wk),
                               np.asarray(Wv), np.asarray(Wo))

    if "nc" not in _NC_CACHE:
        _NC_CACHE["nc"] = build_nc()
    nc = _NC_CACHE["nc"]

    in_maps = make_in_maps(np.asarray(input_tensor), np.asarray(cos),
                           np.asarray(sin), np.asarray(Wq), np.asarray(Wk),
                           np.asarray(Wv), np.asarray(Wo))
    res = run_bass_kernel_spmd(nc, in_maps, core_ids=list(range(NCORES)))
    acc = np.zeros((S, DIN), np.float32)
    for r in res.results:
        acc += np.asarray(r["out"], dtype=np.float32)
    return acc.reshape(1, S, DIN)


# revision 18
# speedup vs baseline: 1.2151x; 1.0190x over previous
"""GroupQueryAttention on 8 TRN2 NeuronCores.

Strategy: tensor-parallel over heads. H=32 query heads, KV=8 kv heads,
group size G=4 -> each core owns exactly 1 kv head and its 4 query heads.
Per core:
  - QKV projections from a replicated (pre-transposed, channels-major) input
  - RoPE on Q/K (rotate-half, done on DVE across partition halves)
  - attention with scores computed TRANSPOSED ([keys, q] layout) so the
    exp(scores) tiles feed the V-matmul directly as the moving operand;
    softmax normalization is deferred: O = V.E, then ctx = O * (1/colsum(E))
  - partial output ctx @ Wo_shard  (row-shard of Wo)
Host sums the 8 partial outputs (the "all-reduce" of the row-parallel Wo).

Perf structure (v3):
  - softmax normalization: colsum matmul (ones) -> DVE reciprocal_approx_fast
    -> rank-1 broadcast matmul -> ACT copy -> DVE scale. Short chain, no
    activation-table thrash (only Exp is ever used on ACT), no long DVE
    iterative divides
  - causal diagonal tiles trimmed: score/PV matmuls only cover the
    not-fully-masked query columns; a single 128x128 lower-tri mask
  - per-head projection emission (K, V, then Q_h right before head h's
    attention) so startup overlaps DMA, and out-projection of quarter q-1
    interleaves into quarter q's attention as PE filler
  - PSUM->SBUF drains split between ACT and DVE; output rows coalesced
    into [128, 2048] tiles before DMA
"""

import sys

sys.path.insert(0, "/opt/trn_rl_repo")

from contextlib import ExitStack

import numpy as np
import ml_dtypes

import concourse.bass as bass
import concourse.bacc as bacc
import concourse.tile as tile
from concourse import mybir
from concourse import bass_isa
from concourse.bass_utils import run_bass_kernel_spmd

BF16 = ml_dtypes.bfloat16

S = 2048          # sequence length
DIN = 4096        # model dim
H, KV, DH = 32, 8, 128
G = H // KV       # 4 query heads per kv head
NCORES = 8
HPC = H // NCORES     # 4 query heads per core
DPC = HPC * DH        # 512 = per-core q-projection width

NQ = 4            # s-quarters (chunks of 512 queries)
QC = S // NQ      # 512
KT = 128          # key tile (partition dim of transposed scores)
NKT = S // KT     # 16 key tiles
NK = DIN // 128   # 32 contraction tiles for projections
NXC = 8           # x chunks per quarter (k-groups of CW)
CW = NK // NXC    # 4 k-tiles per x chunk
SCALE = 1.0 / float(np.sqrt(DH))
EXP_BIAS = -10.0  # constant shift inside exp; cancels in normalization


def build_nc():
    """Build the per-core Bass program (same program on all 8 cores; the
    per-core weight shards arrive via in_maps)."""
    nc = bacc.Bacc()
    dt = mybir.dt

    # ---- DRAM parameters (host-prepared layouts; all DMA-contiguous) ----
    # x[p, sq, k, sc] = x_orig[512*sq + sc, 128*k + p]   (channels-major)
    x = nc.declare_dram_parameter("x", [128, NQ, NK, QC], dt.bfloat16, isOutput=False)
    # wq[p, k, m, d] = Wq_shard[128*k + p, 128*m + d]
    wq = nc.declare_dram_parameter("wq", [128, NK, HPC, DH], dt.bfloat16, isOutput=False)
    # wk[p, k, d] = Wk_shard[128*k + p, d]
    wk = nc.declare_dram_parameter("wk", [128, NK, DH], dt.bfloat16, isOutput=False)
    wv = nc.declare_dram_parameter("wv", [128, NK, DH], dt.bfloat16, isOutput=False)
    # wo[p, h, n] = Wo_shard[128*h + p, n]
    wo = nc.declare_dram_parameter("wo", [128, HPC, DIN], dt.bfloat16, isOutput=False)
    # cosT[d, s] = cos[s, d]; sinm[d, s] = -sin[s, d] for d<64 else +sin[s, d]
    cosT = nc.declare_dram_parameter("cosT", [DH, S], dt.float32, isOutput=False)
    sinm = nc.declare_dram_parameter("sinm", [DH, S], dt.float32, isOutput=False)
    # tri[p, c] = 1.0 if p <= c else 0.0  (128x128 causal triangle)
    tri = nc.declare_dram_parameter("tri", [128, 128], dt.bfloat16, isOutput=False)
    ident = nc.declare_dram_parameter("ident", [128, 128], dt.bfloat16, isOutput=False)
    ones_col = nc.declare_dram_parameter("ones_col", [128, 1], dt.bfloat16, isOutput=False)
    out = nc.declare_dram_parameter("out", [S, DIN], dt.bfloat16, isOutput=True)

    with tile.TileContext(nc) as tc, ExitStack() as ctx:
        singles = ctx.enter_context(tc.tile_pool(name="singles", bufs=1))
        wqp = ctx.enter_context(tc.tile_pool(name="wqp", bufs=1))
        wop = ctx.enter_context(tc.tile_pool(name="wop", bufs=1))
        xp = ctx.enter_context(tc.tile_pool(name="xp", bufs=1))
        qkv = ctx.enter_context(tc.tile_pool(name="qkv", bufs=1))
        epool = ctx.enter_context(tc.tile_pool(name="epool", bufs=4))
        spool = ctx.enter_context(tc.tile_pool(name="spool", bufs=2))
        npool = ctx.enter_context(tc.tile_pool(name="npool", bufs=2))
        tpool = ctx.enter_context(tc.tile_pool(name="tpool", bufs=2))
        obp = ctx.enter_context(tc.tile_pool(name="obp", bufs=3))
        ps_proj = ctx.enter_context(tc.tile_pool(name="ps_proj", bufs=2, space="PSUM"))
        ps_po = ctx.enter_context(tc.tile_pool(name="ps_po", bufs=2, space="PSUM"))
        ps_sc = ctx.enter_context(tc.tile_pool(name="ps_sc", bufs=2, space="PSUM"))
        ps_out = ctx.enter_context(tc.tile_pool(name="ps_out", bufs=2, space="PSUM"))

        # ---- constants / weights resident in SBUF ----
        # DMA emission order == consumption order so the PE never stalls at
        # kernel start: wk + x chunks feed the K-projection, wv + wq_h0
        # right after, cos/sin for RoPE before quarter-0 attention; the
        # remaining q-head weights and wo trail in.
        w_k = singles.tile([128, NK, DH], dt.bfloat16, tag="wk")
        nc.sync.dma_start(out=w_k[:, 0:8], in_=wk[:, 0:8])

        xcs0 = []
        for g in range(NXC):
            xc = xp.tile([128, NK // NXC, QC], dt.bfloat16, tag=f"xc{g}",
                         name=f"xc{g}_0")
            nc.sync.dma_start(out=xc, in_=x[:, 0, g * CW:(g + 1) * CW])
            xcs0.append(xc)
            if g % 2 == 1 and g < NXC - 1:
                gg = (g + 1) // 2
                nc.sync.dma_start(out=w_k[:, 8 * gg:8 * (gg + 1)],
                                  in_=wk[:, 8 * gg:8 * (gg + 1)])

        w_v = singles.tile([128, NK, DH], dt.bfloat16, tag="wv")
        nc.sync.dma_start(out=w_v, in_=wv[:])

        w_q = [wqp.tile([128, NK, DH], dt.bfloat16, tag=f"wq{h}", name=f"wq{h}")
               for h in range(HPC)]
        nc.sync.dma_start(out=w_q[0], in_=wq[:, :, 0])

        c_cos = singles.tile([DH, S], dt.float32, tag="cos")
        nc.sync.dma_start(out=c_cos, in_=cosT[:])
        c_sin = singles.tile([DH, S], dt.float32, tag="sin")
        nc.sync.dma_start(out=c_sin, in_=sinm[:])

        c_tri = singles.tile([128, 128], dt.bfloat16, tag="tri")
        nc.sync.dma_start(out=c_tri, in_=tri[:])
        c_id = singles.tile([128, 128], dt.bfloat16, tag="ident")
        nc.sync.dma_start(out=c_id, in_=ident[:])
        c_oc = singles.tile([128, 1], dt.bfloat16, tag="ones_col")
        nc.sync.dma_start(out=c_oc, in_=ones_col[:])

        for h in range(1, HPC):
            nc.sync.dma_start(out=w_q[h], in_=wq[:, :, h])

        w_o = wop.tile([128, HPC, DIN], dt.bfloat16, tag="wo")
        for h in range(HPC):
            nc.sync.dma_start(out=w_o[:, h], in_=wo[:, h])

        c_bias = singles.tile([128, 1], dt.float32, tag="ebias")
        nc.vector.memset(c_bias, EXP_BIAS)

        # ---- long-lived activations ----
        qt = [qkv.tile([DH, S], dt.bfloat16, tag=f"qt{h}", name=f"qt{h}")
              for h in range(HPC)]
        kt = qkv.tile([DH, S], dt.bfloat16, tag="kt")
        vn = qkv.tile([128, NKT, DH], dt.bfloat16, tag="vn")   # V natural tiles
        ctxT = [qkv.tile([DH, S], dt.bfloat16, tag=f"ctx{h}", name=f"ctx{h}")
                for h in range(HPC)]

        def rope_from_psum(ps, dst_slice, s0):
            """dst = ps*cos + rot_half(ps)*sinm over s-columns [s0, s0+QC)."""
            t1 = tpool.tile([DH, QC], dt.float32, tag="t1", name="t1")
            nc.vector.tensor_mul(t1, ps, c_cos[:, s0:s0 + QC])
            t2 = tpool.tile([DH, QC], dt.float32, tag="t2", name="t2")
            nc.vector.tensor_mul(t2[0:64, :], ps[64:128, :], c_sin[0:64, s0:s0 + QC])
            nc.vector.tensor_mul(t2[64:128, :], ps[0:64, :], c_sin[64:128, s0:s0 + QC])
            nc.vector.tensor_add(dst_slice, t1, t2)

        def emit_kv(sq, xcs):
            """K and V projections + RoPE(K) + V transpose for quarter sq."""
            s0 = sq * QC
            psk = ps_proj.tile([DH, QC], dt.float32, tag="acc", name="psk")
            for k in range(NK):
                nc.tensor.matmul(psk, lhsT=w_k[:, k], rhs=xcs[k // CW][:, k % CW],
                                 start=(k == 0), stop=(k == NK - 1))
            rope_from_psum(psk, kt[:, s0:s0 + QC], s0)

            psv = ps_proj.tile([DH, QC], dt.float32, tag="acc", name="psv")
            for k in range(NK):
                nc.tensor.matmul(psv, lhsT=w_v[:, k], rhs=xcs[k // CW][:, k % CW],
                                 start=(k == 0), stop=(k == NK - 1))
            vtmp = tpool.tile([DH, QC], dt.bfloat16, tag="vtmp", name="vtmp")
            nc.scalar.copy(vtmp, psv)
            for i in range(QC // 128):
                pvt = ps_sc.tile([128, 128], dt.bfloat16, tag="sc", name="pvt")
                nc.tensor.transpose(pvt, vtmp[:, i * 128:(i + 1) * 128], c_id)
                nc.vector.tensor_copy(vn[:, sq * 4 + i], pvt)

        def emit_q(sq, h, xcs):
            """Q projection + RoPE for head h, quarter sq.

            The x-chunk read order is rotated per head so that the last Q
            stream (h3) reads chunk 0 first -> next quarter's chunk-0
            prefetch DMA (WAR on these reads) can start that much sooner,
            staggered in the same order the next quarter consumes them."""
            s0 = sq * QC
            psq = ps_proj.tile([DH, QC], dt.float32, tag="acc", name="psq")
            korder = [CW * ((2 * (h + 1) + g) % NXC) + kk
                      for g in range(NXC) for kk in range(CW)]
            for i, k in enumerate(korder):
                nc.tensor.matmul(psq, lhsT=w_q[h][:, k],
                                 rhs=xcs[k // CW][:, k % CW],
                                 start=(i == 0), stop=(i == NK - 1))
            rope_from_psum(psq, qt[h][:, s0:s0 + QC], s0)

        def emit_attn_head(sq, h):
            """Causal attention for head h over quarter sq's queries.

            Scores are [key-tile, q] transposed; diagonal key-tiles are
            trimmed to the columns that aren't fully masked, and the
            128-wide triangle on the diagonal gets the 0/1 mask."""
            s0 = sq * QC
            njt = 4 * (sq + 1)
            sacc = spool.tile([128, QC], dt.bfloat16, tag="sacc", name="sacc")
            po = ps_po.tile([DH, QC], dt.float32, tag="po", name="po")
            for jt in range(njt):
                r = jt - (njt - 4)          # >=0 -> diagonal tile index
                c0 = 128 * r if r > 0 else 0
                psc = ps_sc.tile([128, QC], dt.float32, tag="sc", name="psc")
                nc.tensor.matmul(psc[:, c0:QC], lhsT=kt[:, jt * KT:(jt + 1) * KT],
                                 rhs=qt[h][:, s0 + c0:s0 + QC],
                                 start=True, stop=True)
                e = epool.tile([128, QC], dt.bfloat16, tag="e", name="e")
                nc.scalar.activation(out=e[:, c0:QC], in_=psc[:, c0:QC],
                                     func=mybir.ActivationFunctionType.Exp,
                                     bias=c_bias, scale=SCALE)
                if r >= 0:
                    nc.vector.tensor_mul(e[:, c0:c0 + 128], e[:, c0:c0 + 128],
                                         c_tri)
                if jt == 0:
                    nc.vector.tensor_copy(sacc, e[:, 0:QC])
                else:
                    nc.vector.tensor_add(sacc[:, c0:QC], sacc[:, c0:QC],
                                         e[:, c0:QC])
                nc.tensor.matmul(po[:, c0:QC], lhsT=vn[:, jt], rhs=e[:, c0:QC],
                                 start=(jt == 0), stop=(jt == njt - 1))
            # normalization: ctx = O * (1/colsum(E)). colsum via a ones
            # matmul, fast approximate reciprocal on DVE (single custom op,
            # ~51 ULP), partition-broadcast on the otherwise-idle gpsimd,
            # DVE scale. Nothing expensive on any engine, no PE wait.
            pcs = ps_out.tile([1, QC], dt.float32, tag="pso", name="pcs")
            nc.tensor.matmul(pcs, lhsT=c_oc, rhs=sacc, start=True, stop=True)
            rec = npool.tile([1, QC], dt.float32, tag="rec", name="rec")
            nc.vector.reciprocal_approx_fast(out=rec, in_=pcs)
            rcs = npool.tile([128, QC], dt.float32, tag="nrm", name="rcs")
            nc.gpsimd.partition_broadcast(rcs, rec)
            nc.vector.tensor_mul(ctxT[h][:, s0:s0 + QC], po, rcs)

        def emit_outproj_block(st):
            """out[st*128:(st+1)*128, :] = sum_h ctxT[h][:, st-block].T @ Wo[h]."""
            for quad in range(4):
                ob = obp.tile([128, DIN // 4], dt.bfloat16, tag="ob", name="ob")
                for j in range(2):
                    oc = quad * 2 + j
                    pso = ps_out.tile([128, 512], dt.float32, tag="pso", name="pso")
                    for h in range(HPC):
                        nc.tensor.matmul(pso,
                                         lhsT=ctxT[h][:, st * 128:(st + 1) * 128],
                                         rhs=w_o[:, h, oc * 512:(oc + 1) * 512],
                                         start=(h == 0), stop=(h == HPC - 1))
                    dst = ob[:, j * 512:(j + 1) * 512]
                    if oc % 2 == 0:
                        nc.scalar.copy(dst, pso)
                    else:
                        nc.vector.tensor_copy(dst, pso)
                nc.sync.dma_start(
                    out=out[st * 128:(st + 1) * 128,
                            quad * (DIN // 4):(quad + 1) * (DIN // 4)],
                    in_=ob)

        # ---- main pipeline ----
        # Per quarter: K/V projections, then per head [Q_h, attention_h,
        # out-proj block of the previous quarter]. The out-proj matmuls are
        # independent PE filler while attention waits on exp results; the
        # x chunks for the next quarter prefetch once the last Q reads them.
        xcs = xcs0
        for sq in range(NQ):
            emit_kv(sq, xcs)
            for h in range(HPC):
                emit_q(sq, h, xcs)
                if h == HPC - 1 and sq + 1 < NQ:
                    nxt = []
                    for g in range(NXC):
                        xc = xp.tile([128, NK // NXC, QC], dt.bfloat16,
                                     tag=f"xc{g}", name=f"xc{g}_{sq + 1}")
                        nc.sync.dma_start(out=xc,
                                          in_=x[:, sq + 1, g * CW:(g + 1) * CW])
                        nxt.append(xc)
                emit_attn_head(sq, h)
                # out-proj blocks of the previous quarter fill PE bubbles at
                # heads 0-2; block 4*sq at the quarter boundary (right after
                # head 3) covers the gap while the next quarter's x arrives.
                if sq >= 1 and h < HPC - 1:
                    emit_outproj_block(4 * (sq - 1) + h + 1)
            emit_outproj_block(4 * sq)
            if sq + 1 < NQ:
                xcs = nxt
        for st in range(4 * (NQ - 1) + 1, 4 * NQ):
            emit_outproj_block(st)
    nc.finalize()
    return nc


def make_in_maps(input_tensor, cos, sin, Wq, Wk, Wv, Wo):
    """Host-side sharding + layout preparation. Returns list of 8 dicts."""
    x2 = np.ascontiguousarray(input_tensor.reshape(S, DIN))
    # x_host[p, sq, k, sc] = x2[512*sq+sc, 128*k+p]
    xt = x2.T.astype(BF16)                      # [DIN, S]
    x_host = np.ascontiguousarray(
        xt.reshape(NK, 128, NQ, QC).transpose(1, 2, 0, 3))

    cosT = np.ascontiguousarray(cos.T.astype(np.float32))
    sinm = np.ascontiguousarray(sin.T.astype(np.float32))
    sinm = sinm.copy()
    sinm[0:64, :] *= -1.0

    p_idx = np.arange(128)[:, None]
    c_idx = np.arange(128)[None, :]
    tri = (p_idx <= c_idx).astype(BF16)

    ident = np.eye(128, dtype=BF16)
    ones_col = np.ones((128, 1), dtype=BF16)

    common = dict(x=x_host, cosT=cosT, sinm=sinm, tri=tri, ident=ident,
                  ones_col=ones_col)

    in_maps = []
    for c in range(NCORES):
        wq_s = Wq[:, c * DPC:(c + 1) * DPC].astype(BF16)
        wq_host = np.ascontiguousarray(
            wq_s.reshape(NK, 128, HPC, DH).transpose(1, 0, 2, 3))
        wk_s = Wk[:, c * DH:(c + 1) * DH].astype(BF16)
        wk_host = np.ascontiguousarray(wk_s.reshape(NK, 128, DH).transpose(1, 0, 2))
        wv_s = Wv[:, c * DH:(c + 1) * DH].astype(BF16)
        wv_host = np.ascontiguousarray(wv_s.reshape(NK, 128, DH).transpose(1, 0, 2))
        wo_s = Wo[c * DPC:(c + 1) * DPC, :].astype(BF16)
        wo_host = np.ascontiguousarray(wo_s.reshape(HPC, 128, DIN).transpose(1, 0, 2))
        in_maps.append(dict(common, wq=wq_host, wk=wk_host, wv=wv_host, wo=wo_host))
    return in_maps


def _numpy_fallback(input_tensor, attention_mask, cos, sin, Wq, Wk, Wv, Wo):
    x = input_tensor.astype(np.float32)
    b, s, _ = x.shape
    q = (x @ Wq).reshape(b, s, H, DH).transpose(0, 2, 1, 3)
    k = (x @ Wk).reshape(b, s, KV, DH).transpose(0, 2, 1, 3)
    v = (x @ Wv).reshape(b, s, KV, DH).transpose(0, 2, 1, 3)

    def rope(t):
        t1, t2 = t[..., :64], t[..., 64:]
        rot = np.concatenate([-t2, t1], axis=-1)
        return t * cos[None, None] + rot * sin[None, None]

    q, k = rope(q), rope(k)
    k = np.repeat(k, G, axis=1)
    v = np.repeat(v, G, axis=1)
    sc = np.einsum('bhqd,bhkd->bhqk', q, k)
    sc = np.where(attention_mask, -np.inf, sc) / np.float32(np.sqrt(DH))
    sc = sc - sc.max(axis=-1, keepdims=True)
    w = np.exp(sc)
    w = w / w.sum(axis=-1, keepdims=True)
    ctx = np.einsum('bhqk,bhkd->bhqd', w, v)
    ctx = ctx.transpose(0, 2, 1, 3).reshape(b, s, H * DH)
    return (ctx @ Wo).astype(np.float32)


_NC_CACHE = {}


def kernel(input_tensor, attention_mask, cos, sin, Wq, Wk, Wv, Wo):
    mask = np.asarray(attention_mask).reshape(S, S)
    causal = np.array_equal(mask, np.triu(np.ones((S, S), bool), k=1))
    if not causal:
        return _numpy_fallback(np.asarray(input_tensor), np.asarray(attention_mask),
                               np.asarray(cos), np.asarray(sin),
                               np.asarray(Wq), np.asarray(Wk),
                               np.asarray(Wv), np.asarray(Wo))

    if "nc" not in _NC_CACHE:
        _NC_CACHE["nc"] = build_nc()
    nc = _NC_CACHE["nc"]

    in_maps = make_in_maps(np.asarray(input_tensor), np.asarray(cos),
                           np.asarray(sin), np.asarray(Wq), np.asarray(Wk),
                           np.asarray(Wv), np.asarray(Wo))
    res = run_bass_kernel_spmd(nc, in_maps, core_ids=list(range(NCORES)))
    acc = np.zeros((S, DIN), np.float32)
    for r in res.results:
        acc += np.asarray(r["out"], dtype=np.float32)
    return acc.reshape(1, S, DIN)
